# revision 1
# baseline (speedup 1.0000x reference)
"""Trainium2 Bass kernel for nn_AttentionModule_7146825580577.

Strategy (see spec sharding_hint): pure data parallel over the batch dim
(8192 rows -> 1024 rows per core, 8 cores), weights replicated.

Device math (per core), in feature-transposed layout (features on SBUF
partitions, batch on the free dim), fp32 data with float32r matmuls:

  - All LayerNorms whose input is an affine function of a previous
    activation use host-side column-centered weights, so mean(y) == 0 by
    construction and only sum(y^2) is needed on device (computed by a
    ones-vector matmul on the PE, reduced over partitions).
  - seq_len==1 MHA reduces to out_proj(v_proj(kv)); both projections are
    fused on the host into a single 512x512 effective matrix. The self-
    attention residual (x + sa(x)) is folded into a single matmul with
    weights I + Wv@Wo.
  - The cross-attention pair average (a+b)/2 is a single concat-matmul.
  - The n2 LayerNorm (after gating) is folded into the fus_W1 matmul:
    gamma scales fold into the weights, the per-sample mean correction is
    a rank-1 matmul term, betas fold into the bias.
  - 1/sqrt(var+eps) is computed on the vector engine with the int32 bit
    trick + Newton-Raphson iterations, on PE-transposed [128, k] stat
    tiles so each op touches only a tiny free dim.
  - Input hidden states / logits are transposed on the host (numpy) so no
    on-device transposes are needed; the output is produced transposed
    and transposed back on the host.
"""
import os
import sys

sys.path.insert(0, "/opt/trn_rl_repo")

import numpy as np

import concourse.bass as bass
import concourse.tile as tile
from concourse import bacc, mybir
from concourse.bass import ts
from concourse.bass_utils import run_bass_kernel_spmd
from concourse.masks import make_identity

D = 512
HID = 1024
B = 8192
NCORES = 8
BL = B // NCORES          # rows per core
NBT = BL // D             # batch tiles per core (2)
EPS = 1e-5
MAGIC = 0x5F3759DF
F32 = mybir.dt.float32
I32 = mybir.dt.int32
FS = [10, 6, 15]          # logit dims per stream
NR_ITERS = int(os.environ.get("KERNEL_NR_ITERS", "2"))
MM_DT = {
    "f32r": mybir.dt.float32r,
    "f32": mybir.dt.float32,
}[os.environ.get("KERNEL_MM_DTYPE", "f32r")]

F64 = np.float64


# --------------------------------------------------------------------------
# Host-side weight folding
# --------------------------------------------------------------------------

def _center_cols(W, b):
    W = np.asarray(W, F64)
    b = np.asarray(b, F64)
    return W - W.mean(axis=1, keepdims=True), b - b.mean()


def fold_weights(inp):
    g = lambda k: np.asarray(inp[k], dtype=F64)
    out = {}

    w_hp, b_hp = [], []
    for s in range(3):
        W, b = _center_cols(g("hp_W")[s], g("hp_b")[s])
        w_hp.append(W)
        b_hp.append(b)
    out["w_hp"] = np.stack(w_hp)
    out["b_hp"] = np.stack(b_hp)
    out["g_hp"], out["be_hp"] = g("hp_g"), g("hp_be")

    mhaW, mhab = g("mha_in_W"), g("mha_in_b")
    moW, mob = g("mha_out_W"), g("mha_out_b")
    Wv0, bv0 = mhaW[0][:, 2 * D:], mhab[0][2 * D:]
    Wr, br = _center_cols(np.eye(D) + Wv0 @ moW[0], bv0 @ moW[0] + mob[0])
    out["w_r"], out["b_r"] = Wr, br
    out["g_n1"], out["be_n1"] = g("n1_g"), g("n1_be")

    Wj, bj = [None] * 4, [None] * 4
    for j in (1, 2, 3):
        Wv, bv = mhaW[j][:, 2 * D:], mhab[j][2 * D:]
        Wj[j] = Wv @ moW[j]
        bj[j] = bv @ moW[j] + mob[j]
    # m_verb uses (inst_e @ W1, target_e @ W2); m_inst (verb @ W1, target @ W3);
    # m_target (verb @ W2, inst @ W3)
    mods = [(1, 2), (1, 3), (2, 3)]
    out["m_streams"] = [(1, 2), (0, 2), (0, 1)]
    w_m, b_m = [], []
    for s in range(3):
        ja, jb = mods[s]
        w_m.append(np.concatenate([0.5 * Wj[ja], 0.5 * Wj[jb]], axis=0))
        b_m.append(0.5 * (bj[ja] + bj[jb]))
    out["w_m"] = np.stack(w_m)
    out["b_m"] = np.stack(b_m)

    out["w_g"] = g("gate_W")
    out["b_g"] = g("gate_b")

    w_lp, b_lp = [], []
    for s, key in enumerate(["verb", "inst", "target"]):
        W, b = _center_cols(g(f"lp_W_{key}"), g(f"lp_b_{key}"))
        w_lp.append(W)
        b_lp.append(b)
    out["w_lp"] = w_lp
    out["b_lp"] = np.stack(b_lp)
    out["g_lp"], out["be_lp"] = g("lp_g"), g("lp_be")

    W1 = g("fus_W1")
    g2, be2 = g("n2_g"), g("n2_be")
    A1, negc = [], []
    bias_total = g("fus_b1").copy()
    for s in range(3):
        blk = W1[s * D:(s + 1) * D]
        A = g2[s][:, None] * blk
        c = blk.T @ g2[s]
        A1.append(A - A.mean(axis=1, keepdims=True))
        negc.append(-(c - c.mean()))
        bias_total += be2[s] @ blk
    L1 = []
    for s in range(3):
        off = 3 * D + s * (D // 2)
        blk = W1[off: off + D // 2]
        L1.append(blk - blk.mean(axis=1, keepdims=True))
    out["w_f1"] = np.stack(A1)
    out["negc_f1"] = np.stack(negc)
    out["w_f1l"] = np.stack(L1)
    out["b_f1"] = bias_total - bias_total.mean()
    out["g_f1"], out["be_f1"] = g("fus_g1"), g("fus_ge1")

    W2c, b2c = _center_cols(g("fus_W2"), g("fus_b2"))
    out["w_f2"], out["b_f2"] = W2c, b2c
    out["g_f2"], out["be_f2"] = g("fus_g2"), g("fus_ge2")
    return out


def _vec_pp(v, nk):
    """[.., nk*128] feature vector -> ACT per-partition layout [.., 128, nk]."""
    v = np.asarray(v, np.float32)
    return np.ascontiguousarray(v.reshape(v.shape[:-1] + (nk, 128)).swapaxes(-1, -2))


def device_arrays(fw):
    """Folded weights -> dict of fp32 arrays matching the DRAM tensor decls."""
    f32 = lambda v: np.ascontiguousarray(np.asarray(v, np.float32))
    dev = {}
    dev["w_hp"] = f32(fw["w_hp"].reshape(3, 8, 128, 512))
    dev["b_hp"] = _vec_pp(fw["b_hp"], 4)
    dev["w_r"] = f32(fw["w_r"].reshape(4, 128, 512))
    dev["b_r"] = _vec_pp(fw["b_r"], 4)
    dev["w_m"] = f32(fw["w_m"].reshape(3, 8, 128, 512))
    dev["b_m"] = _vec_pp(fw["b_m"], 4)
    dev["w_g"] = f32(fw["w_g"].reshape(3, 8, 128, 512))
    for s in range(3):
        dev[f"w_lp{s}"] = f32(fw["w_lp"][s])
    dev["b_lp"] = _vec_pp(fw["b_lp"], 2)
    dev["w_f1"] = f32(fw["w_f1"].reshape(3, 4, 128, 512))
    dev["w_f1l"] = f32(fw["w_f1l"].reshape(3, 2, 128, 512))
    dev["negc_f1"] = f32(fw["negc_f1"][None])
    dev["b_f1"] = _vec_pp(fw["b_f1"], 4)
    dev["w_f2"] = f32(fw["w_f2"].reshape(4, 128, 512))
    dev["b_f2"] = _vec_pp(fw["b_f2"], 4)
    for name in ("g_hp", "be_hp", "g_n1", "be_n1", "b_g"):
        dev[name] = _vec_pp(fw[name], 4)
    dev["g_lp"] = _vec_pp(fw["g_lp"], 2)
    dev["be_lp"] = _vec_pp(fw["be_lp"], 2)
    for name in ("g_f1", "be_f1", "g_f2", "be_f2"):
        dev[name] = _vec_pp(fw[name], 4)
    dev["ones_row"] = np.ones((1, 128), np.float32)
    dev["ones_col"] = np.ones((128, 1), np.float32)
    return dev


# --------------------------------------------------------------------------
# Device program
# --------------------------------------------------------------------------

class _Emit:
    def __init__(self, tc, io):
        self.tc = tc
        self.nc = tc.nc
        self.io = io
        self.ctx = None
        self.flip = 0

    def alt(self):
        """Alternate DVE / ACT for plain copies and squares."""
        self.flip ^= 1
        return self.flip

    def copy(self, out, in_, bias=None):
        """PSUM -> SBUF eviction, optionally adding a per-partition [128,1]
        bias column (the layer bias in transposed layout)."""
        nc = self.nc
        if self.alt():
            if bias is None:
                nc.vector.tensor_copy(out, in_)
            else:
                nc.vector.tensor_scalar_add(out, in_, bias)
        else:
            if bias is None:
                nc.scalar.activation(out, in_,
                                     mybir.ActivationFunctionType.Copy)
            else:
                nc.scalar.activation(out, in_,
                                     mybir.ActivationFunctionType.Identity,
                                     bias=bias)

    def square(self, out, in_sbuf, in_psum):
        """Square either from the evicted SBUF copy (DVE) or PSUM (ACT)."""
        nc = self.nc
        if self.alt():
            nc.vector.tensor_mul(out, in_sbuf, in_sbuf)
        else:
            nc.scalar.activation(out, in_psum,
                                 mybir.ActivationFunctionType.Square)


MF = MM_DT  # dtype of every tensor consumed by a matmul


def _rd(ap):
    return ap


DEBUG = bool(os.environ.get("KERNEL_DEBUG"))


def emit_program(tc, io):
    nc = tc.nc

    def dbg(name, tile_ap):
        if DEBUG and name in io:
            nc.sync.dma_start(io[name], tile_ap)
    from contextlib import ExitStack
    ctx = ExitStack()
    em = _Emit(tc, io)
    ACT = mybir.ActivationFunctionType

    # ---------------- pools ----------------
    P = lambda name, bufs, space="SBUF": ctx.enter_context(
        tc.tile_pool(name=name, bufs=bufs, space=space))
    const = P("const", 1)
    wpool = P("wchunk", 3)
    xpool = P("xchunk", 2)
    evp = P("ev", 10)
    sqp = P("sq", 2)
    zp = P("z", 2)
    yhp = P("yh", 1)
    ep = P("e", 3)
    mp = P("m", 2)
    sgp = P("sg", 1)
    qp = P("q", 1)
    tp = P("t", 2)
    ztp = P("zt", 3)
    lp_ = P("l", 3)
    hp_ = P("h", 1)
    op_ = P("o", 1)
    stp = P("stats_sb", 9)
    bcp = P("bc_sb", 2)
    ltp = P("lt", 1)
    mm_ps = P("mm_ps", 4, "PSUM")
    st_ps = P("st_ps", 4, "PSUM")

    # ---------------- constants / resident weights ----------------
    ident = const.tile([128, 128], F32)
    make_identity(nc, ident)
    ones_row = const.tile([1, 128], MF)
    nc.sync.dma_start(ones_row[:], io["ones_row"])
    ones_col = const.tile([128, 1], MF)
    nc.sync.dma_start(ones_col[:], io["ones_col"])

    def load(name, shape, rearr=None, dtype=F32):
        t = const.tile(shape, dtype, name=name)
        src = io[name]
        if rearr:
            src = src.rearrange(rearr)
        nc.sync.dma_start(t[:], src)
        return t

    b_hp = load("b_hp", [128, 3, 4], "s p c -> p s c")
    b_r = load("b_r", [128, 4])
    b_m = load("b_m", [128, 3, 4], "s p c -> p s c")
    b_lp = load("b_lp", [128, 3, 2], "s p c -> p s c")
    negc = load("negc_f1", [1, 3, 512], dtype=MF)
    b_f1 = load("b_f1", [128, 4])
    b_f2 = load("b_f2", [128, 4])
    g_hp = load("g_hp", [128, 3, 4], "s p c -> p s c")
    be_hp = load("be_hp", [128, 3, 4], "s p c -> p s c")
    g_n1 = load("g_n1", [128, 3, 4], "s p c -> p s c")
    be_n1 = load("be_n1", [128, 3, 4], "s p c -> p s c")
    b_g = load("b_g", [128, 3, 4], "s p c -> p s c")
    g_lp = load("g_lp", [128, 3, 2], "s p c -> p s c")
    be_lp = load("be_lp", [128, 3, 2], "s p c -> p s c")
    g_f1 = load("g_f1", [128, 4])
    be_f1 = load("be_f1", [128, 4])
    g_f2 = load("g_f2", [128, 4])
    be_f2 = load("be_f2", [128, 4])
    w_lp = [load(f"w_lp{s}", [FS[s], 256], dtype=MF) for s in range(3)]

    # ---------------- helpers ----------------
    def emit_istd(v_sb, k):
        """v_sb: [k,512] sbuf fp32 variances (+eps already added).
        Returns list of k istd row tiles [1,512] (MF), via PE-transposed
        Newton-Raphson rsqrt (int32 magic seed)."""
        vT = st_ps.tile([128, 4 * k], F32, name="vT", tag="stat_ps")
        for c in range(4):
            nc.tensor.transpose(vT[:, c * k:(c + 1) * k],
                                v_sb[0:k, ts(c, 128)], ident[0:k, 0:k])
        y = stp.tile([128, 4 * k], F32, name="nr_y", tag="ssb")
        t = stp.tile([128, 4 * k], F32, name="nr_t", tag="ssb")
        nc.vector.tensor_scalar(y[:].bitcast(I32), vT[:].bitcast(I32),
                                1, None, mybir.AluOpType.logical_shift_right)
        nc.vector.tensor_scalar(y[:].bitcast(I32), y[:].bitcast(I32),
                                -1, MAGIC, mybir.AluOpType.mult,
                                mybir.AluOpType.add)
        for _ in range(NR_ITERS):
            nc.vector.tensor_mul(t[:], y[:], y[:])
            nc.vector.tensor_mul(t[:], t[:], vT[:])
            nc.vector.tensor_scalar(t[:], t[:], -0.5, 1.5,
                                    mybir.AluOpType.mult, mybir.AluOpType.add)
            nc.vector.tensor_mul(y[:], y[:], t[:])
        rows = []
        for s in range(k):
            rT = st_ps.tile([1, 512], F32, name="rT", tag="stat_ps")
            for c in range(4):
                nc.tensor.transpose(rT[0:1, ts(c, 128)],
                                    y[:, c * k + s:c * k + s + 1], ident)
            istd = stp.tile([1, 512], MF, name="istd", tag="ssb")
            nc.vector.tensor_copy(istd[:], rT[:])
            rows.append(istd)
        return rows

    def bcast(row_ap):
        """[1,512] sbuf row -> [128,512] sbuf tile via GPSIMD."""
        bc = bcp.tile([128, 512], MF, name="bc")
        nc.gpsimd.partition_broadcast(bc[:], row_ap)
        return bc

    def emit_ln(ps_list, bias_cols=None):
        """Evict psum chunks to SBUF (adding the layer bias per partition)
        and accumulate sum(y^2) into a [1,512] psum row."""
        nch = len(ps_list)
        ev = []
        for c, ps in enumerate(ps_list):
            e = evp.tile([128, 512], F32, name="ev")
            em.copy(e[:], ps[:], None if bias_cols is None else bias_cols[c])
            ev.append(e)
        st = st_ps.tile([1, 512], F32, name="st", tag="stat_ps")
        for c in range(nch):
            sq = sqp.tile([128, 512], MF, name="sq")
            em.square(sq[:], ev[c][:], ev[c][:])
            nc.tensor.matmul(st[:], ones_col[:], sq[:],
                             start=(c == 0), stop=(c == nch - 1))
        return ev, st

    def ln_finish(ev, st, gam, bet, func, out_tile, dim=D):
        v = stp.tile([1, 512], F32, name="v", tag="ssb")
        nc.vector.tensor_scalar(v[0:1, :], st[:], 1.0 / dim, EPS,
                                mybir.AluOpType.mult, mybir.AluOpType.add)
        istd = emit_istd(v, 1)[0]
        bc = bcast(istd[:])
        for c, e in enumerate(ev):
            z = zp.tile([128, 512], F32, name="z")
            nc.vector.tensor_mul(z[:], e[:], bc[:])
            nc.scalar.activation(out_tile[:, c, :], z[:], func,
                                 bias=bet[:, c:c + 1], scale=gam[:, c:c + 1])

    def mm_group(n_m, srcs, bias_cols):
        """Emit an accumulating matmul group. srcs = list of (lhsT_fn, rhs)
        k-chunks; returns (ev, st) after evict+square+stats."""
        ps_list = [mm_ps.tile([128, 512], F32, name="mm") for _ in range(n_m)]
        last = len(srcs) - 1
        for ci, (lhsT_fn, rhs) in enumerate(srcs):
            for m in range(n_m):
                nc.tensor.matmul(ps_list[m][:], lhsT_fn(m), rhs,
                                 start=(ci == 0), stop=(ci == last))
        return ps_list

    # ---------------- main ----------------
    pend = []

    def flush(n=None):
        cnt = len(pend) if n is None else n
        for _ in range(cnt):
            if pend:
                pend.pop(0)()

    def wchunk(dram_ap):
        wc = wpool.tile([128, 512], MF, name="wc", tag="wc1")
        nc.sync.dma_start(wc[:], dram_ap)
        return wc

    def wpair(dram_pair_ap):
        """Load two [128,512] k-chunks in one DMA -> [128,2,512] tile."""
        wc = wpool.tile([128, 2, 512], MF, name="wcp", tag="wcp")
        nc.sync.dma_start(wc[:], dram_pair_ap.rearrange("c p n -> p c n"))
        return wc

    def pair_srcs(dram_4d, nk, rhs_fn):
        srcs = []
        for c0 in range(0, nk, 2):
            wc = wpair(dram_4d[c0:c0 + 2])
            for cc in range(2):
                srcs.append((lambda m, wc=wc, cc=cc: wc[:, cc, ts(m, 128)],
                             rhs_fn(c0 + cc)))
        return srcs

    for bt in range(NBT):
        bsl = ts(bt, 512)
        l_tiles = [None] * 3
        e_tiles = [None] * 3
        m_tiles = [None] * 3
        zt_tiles = [None] * 3
        w_rows = [None] * 3
        yh_tiles = [None] * 3

        # ---- lp matmuls (tiny) ----
        def emit_lp_mm(s):
            lt = ltp.tile([FS[s], 512], F32, name="lt", tag="lt")
            nc.sync.dma_start(lt[:], io[f"lT{s}"][:, bsl])
            lsg = ltp.tile([FS[s], 512], MF, name="lsg", tag="lsg")
            nc.scalar.activation(lsg[:], lt[:], ACT.Sigmoid)
            ps_list = [mm_ps.tile([128, 512], F32, name="mm") for _ in range(2)]
            for m in range(2):
                nc.tensor.matmul(ps_list[m][:], w_lp[s][:, ts(m, 128)],
                                 lsg[:], start=True, stop=True)
            ev, st = emit_ln(ps_list, [b_lp[:, s, c:c + 1] for c in range(2)])

            def fin(s=s, ev=ev, st=st):
                l_sb = lp_.tile([128, 2, 512], MF, name="l_sb")
                ln_finish(ev, st, g_lp[:, s], be_lp[:, s], ACT.Gelu, l_sb,
                          dim=D // 2)
                l_tiles[s] = l_sb
            pend.append(fin)

        def emit_hp_mm(s):
            xcs = []
            for c0 in range(0, 8, 2):
                xc = xpool.tile([128, 2, 512], MF, name="xc")
                nc.sync.dma_start(
                    xc[:], io[f"xT{s}"][ts(c0 // 2, 256), bsl].rearrange(
                        "(c p) b -> p c b", p=128))
                xcs.append(xc)
            srcs = pair_srcs(io["w_hp"][s], 8,
                             lambda c: xcs[c // 2][:, c % 2, :])
            ps_list = mm_group(4, srcs, None)
            ev, st = emit_ln(ps_list, [b_hp[:, s, c:c + 1] for c in range(4)])

            def fin(s=s, ev=ev, st=st):
                yh = yhp.tile([128, 4, 512], MF, name="yh")
                ln_finish(ev, st, g_hp[:, s], be_hp[:, s], ACT.Gelu, yh)
                yh_tiles[s] = yh
            pend.append(fin)

        def emit_r_mm(s):
            yh = yh_tiles[s]
            srcs = pair_srcs(io["w_r"], 4, lambda c: yh[:, c, :])
            ps_list = mm_group(4, srcs, None)
            ev, st = emit_ln(ps_list, [b_r[:, c:c + 1] for c in range(4)])

            def fin(s=s, ev=ev, st=st):
                e_sb = ep.tile([128, 4, 512], MF, name="e_sb")
                ln_finish(ev, st, g_n1[:, s], be_n1[:, s], ACT.Identity, e_sb)
                e_tiles[s] = e_sb
            pend.append(fin)

        m_streams = [(1, 2), (0, 2), (0, 1)]

        def emit_m_mm(s):
            sa, sb = m_streams[s]
            srcs = pair_srcs(io["w_m"][s], 8,
                             lambda ci: (e_tiles[sa][:, ci, :] if ci < 4
                                         else e_tiles[sb][:, ci - 4, :]))
            ps_list = mm_group(4, srcs, None)
            m_sb = mp.tile([128, 4, 512], MF, name="m_sb")
            for c in range(4):
                em.copy(m_sb[:, c, :], ps_list[c][:], b_m[:, s, c:c + 1])
            m_tiles[s] = m_sb

        def emit_gate_fuse(s):
            srcs = pair_srcs(io["w_g"][s], 8,
                             lambda ci: (e_tiles[s][:, ci, :] if ci < 4
                                         else m_tiles[s][:, ci - 4, :]))
            ps_list = mm_group(4, srcs, None)
            t_sb = tp.tile([128, 4, 512], MF, name="t_sb")
            for c in range(4):
                sg = sgp.tile([128, 512], F32, name="sg")
                nc.scalar.activation(sg[:], ps_list[c][:], ACT.Sigmoid,
                                     bias=b_g[:, s, c:c + 1])
                q = qp.tile([128, 512], F32, name="q")
                nc.vector.tensor_mul(q[:], sg[:], m_tiles[s][:, c, :])
                nc.vector.tensor_add(t_sb[:, c, :], e_tiles[s][:, c, :], q[:])
            st_sum = st_ps.tile([1, 512], F32, name="st_sum", tag="stat_ps")
            st_sq = st_ps.tile([1, 512], F32, name="st_sq", tag="stat_ps")
            for c in range(4):
                nc.tensor.matmul(st_sum[:], ones_col[:], t_sb[:, c, :],
                                 start=(c == 0), stop=(c == 3))
            for c in range(4):
                sq = sqp.tile([128, 512], MF, name="sq")
                em.square(sq[:], t_sb[:, c, :], t_sb[:, c, :])
                nc.tensor.matmul(st_sq[:], ones_col[:], sq[:],
                                 start=(c == 0), stop=(c == 3))

            def fin(s=s, t_sb=t_sb, st_sum=st_sum, st_sq=st_sq):
                mu = stp.tile([1, 512], F32, name="mu", tag="ssb")
                nc.vector.tensor_scalar_mul(mu[:], st_sum[:], 1.0 / D)
                ev2 = stp.tile([1, 512], F32, name="ev2", tag="ssb")
                nc.vector.tensor_scalar(ev2[:], st_sq[:], 1.0 / D, EPS,
                                        mybir.AluOpType.mult,
                                        mybir.AluOpType.add)
                v = stp.tile([1, 512], F32, name="v", tag="ssb")
                nc.vector.tensor_mul(v[:], mu[:], mu[:])
                nc.vector.tensor_sub(v[:], ev2[:], v[:])
                istd = emit_istd(v, 1)[0]
                w_row = stp.tile([1, 512], MF, name="w_row", tag="ssb")
                nc.vector.tensor_mul(w_row[:], mu[:], istd[:])
                w_rows[s] = w_row
                bc = bcast(istd[:])
                zt = ztp.tile([128, 4, 512], MF, name="zt")
                for c in range(4):
                    nc.vector.tensor_mul(zt[:, c, :], t_sb[:, c, :], bc[:])
                zt_tiles[s] = zt
            pend.append(fin)

        # ---------- emission schedule (software pipelined) ----------
        emit_lp_mm(0)
        emit_lp_mm(1)
        emit_lp_mm(2)
        emit_hp_mm(0)          # hp0 matmuls cover lp NR chains
        flush(2)               # lp0, lp1 fins
        emit_hp_mm(1)
        flush(2)               # lp2 fin + hp0 fin (covered by hp1 matmuls)
        emit_r_mm(0)
        flush(1)               # hp1 fin (covered by r'0/hp1 matmuls)
        emit_hp_mm(2)
        emit_r_mm(1)
        flush(1)               # n1_0 fin -> e0
        flush(1)               # hp2 fin -> yh2
        emit_r_mm(2)
        flush(1)               # n1_1 fin -> e1
        emit_m_mm(2)           # m_target needs e0,e1
        flush(1)               # n1_2 fin -> e2
        emit_gate_fuse(2)
        emit_m_mm(1)           # m_inst needs e0,e2
        flush(1)               # n2_2 fin -> zt2 (covered by m1 matmuls)
        emit_gate_fuse(1)
        emit_m_mm(0)           # m_verb needs e1,e2
        flush(1)               # n2_1 fin -> zt1 (covered by m0 matmuls)
        emit_gate_fuse(0)
        flush(1)               # n2_0 fin -> zt0 (covered by fus1 l/zt2/zt1)

        # ---- fus1: order k-chunks so zt0 (finished last) is consumed last
        srcs = []
        for s in range(3):
            srcs += pair_srcs(io["w_f1l"][s], 2,
                              lambda c, s=s: l_tiles[s][:, c, :])
        for s in (2, 1, 0):
            srcs += pair_srcs(io["w_f1"][s], 4,
                              lambda c, s=s: zt_tiles[s][:, c, :])
        for s in (2, 1, 0):
            srcs.append((lambda m, s=s: negc[0:1, s, ts(m, 128)],
                         w_rows[s][:]))
        ps_list = mm_group(4, srcs, None)
        ev, st = emit_ln(ps_list, [b_f1[:, c:c + 1] for c in range(4)])

        def fin_f1(ev=ev, st=st):
            h_sb = hp_.tile([128, 4, 512], MF, name="h_sb")
            ln_finish(ev, st, g_f1, be_f1, ACT.Gelu, h_sb)
            fin_f1.h = h_sb
        pend.append(fin_f1)
        flush(1)

        # ---- fus2
        h_sb = fin_f1.h
        srcs = pair_srcs(io["w_f2"], 4, lambda c: h_sb[:, c, :])
        ps_list = mm_group(4, srcs, None)
        ev, st = emit_ln(ps_list, [b_f2[:, c:c + 1] for c in range(4)])

        def fin_f2(ev=ev, st=st, bsl=bsl):
            o_sb = op_.tile([128, 4, 512], F32, name="o_sb")
            ln_finish(ev, st, g_f2, be_f2, ACT.Identity, o_sb)
            nc.sync.dma_start(
                io["outT"].rearrange("(c p) b -> p c b", p=128)[:, :, bsl],
                o_sb[:])
        pend.append(fin_f2)
        flush(1)

    flush()
    ctx.close()


def build_program():
    nc = bacc.Bacc("TRN2", target_bir_lowering=False, debug=False,
                   num_devices=NCORES)
    io = {}

    def din(name, shape, dtype=F32):
        io[name] = nc.dram_tensor(name, list(shape), dtype,
                                  kind="ExternalInput").ap()

    for s in range(3):
        din(f"xT{s}", (HID, BL), dtype=MM_DT)
        din(f"lT{s}", (FS[s], BL))
    din("w_hp", (3, 8, 128, 512), dtype=MM_DT)
    din("b_hp", (3, 128, 4))
    din("w_r", (4, 128, 512), dtype=MM_DT)
    din("b_r", (128, 4))
    din("w_m", (3, 8, 128, 512), dtype=MM_DT)
    din("b_m", (3, 128, 4))
    din("w_g", (3, 8, 128, 512), dtype=MM_DT)
    for s in range(3):
        din(f"w_lp{s}", (FS[s], 256), dtype=MM_DT)
    din("b_lp", (3, 128, 2))
    din("w_f1", (3, 4, 128, 512), dtype=MM_DT)
    din("w_f1l", (3, 2, 128, 512), dtype=MM_DT)
    din("negc_f1", (1, 3, 512), dtype=MM_DT)
    din("b_f1", (128, 4))
    din("w_f2", (4, 128, 512), dtype=MM_DT)
    din("b_f2", (128, 4))
    for name in ("g_hp", "be_hp", "g_n1", "be_n1", "b_g"):
        din(name, (3, 128, 4))
    for name in ("g_lp", "be_lp"):
        din(name, (3, 128, 2))
    for name in ("g_f1", "be_f1", "g_f2", "be_f2"):
        din(name, (128, 4))
    din("ones_row", (1, 128), dtype=MM_DT)
    din("ones_col", (128, 1), dtype=MM_DT)
    io["outT"] = nc.dram_tensor("outT", [D, BL], F32,
                                kind="ExternalOutput").ap()
    if os.environ.get("KERNEL_DEBUG"):
        for s in range(3):
            for nm, shp in [(f"dbg_istd_hp{s}", [1, 512]),
                            (f"dbg_yh{s}", [128, 4, 512]),
                            (f"dbg_e{s}", [128, 4, 512]),
                            (f"dbg_l{s}", [128, 2, 512]),
                            (f"dbg_m{s}", [128, 4, 512]),
                            (f"dbg_t{s}", [128, 4, 512]),
                            (f"dbg_w{s}", [1, 512])]:
                io[nm] = nc.dram_tensor(nm, shp, F32,
                                        kind="ExternalOutput").ap()
        io["dbg_h"] = nc.dram_tensor("dbg_h", [128, 4, 512], F32,
                                     kind="ExternalOutput").ap()

    with tile.TileContext(nc) as tc:
        emit_program(tc, io)
    nc.compile()
    return nc


def make_in_maps(inputs):
    fw = fold_weights(inputs)
    dev = device_arrays(fw)
    hidden = [np.asarray(inputs["verb_hidden"], np.float32),
              np.asarray(inputs["inst_hidden"], np.float32),
              np.asarray(inputs["target_hidden"], np.float32)]
    logits = [np.asarray(inputs["verb_logits"], np.float32),
              np.asarray(inputs["inst_logits"], np.float32),
              np.asarray(inputs["target_logits"], np.float32)]
    in_maps = []
    for core in range(NCORES):
        rows = slice(core * BL, (core + 1) * BL)
        m = dict(dev)
        for s in range(3):
            m[f"xT{s}"] = np.ascontiguousarray(hidden[s][rows].T)
            m[f"lT{s}"] = np.ascontiguousarray(logits[s][rows].T)
        in_maps.append(m)
    return in_maps


_NC_CACHE = None


def _run(inputs, **spmd_kwargs):
    global _NC_CACHE
    if _NC_CACHE is None:
        _NC_CACHE = build_program()
    nc = _NC_CACHE
    in_maps = make_in_maps(inputs)
    res = run_bass_kernel_spmd(nc, in_maps, list(range(NCORES)),
                               **spmd_kwargs)
    out = np.empty((B, D), dtype=np.float32)
    for core in range(NCORES):
        out[core * BL:(core + 1) * BL] = res.results[core]["outT"].T
    return out, res


def kernel(**inputs) -> np.ndarray:
    return _run(inputs)[0]


def kernel_profiled(inputs, tmpdir=None):
    """Returns (out, BassKernelResults) with an NTFF-based profile."""
    return _run(inputs, trace=True, tmpdir=tmpdir)



# revision 21
# speedup vs baseline: 1.1334x; 1.1334x over previous
"""Trainium2 Bass kernel for nn_AttentionModule_7146825580577.

Strategy: pure data parallel over the batch dim (8192 rows -> 1024 rows per
core, 8 cores), weights replicated.

Device math (per core), feature-transposed layout (features on SBUF
partitions, batch on the free dim), fp16 matmul operands with fp32 PSUM
accumulation:

  - All LayerNorm gammas are folded into the producing matmul's columns
    host-side; the variance is recovered with per-chunk 1/gamma^2 columns in
    the stats matmul.  Column-centered weights make mean(y) == 0 so only
    sum(y^2) is needed.
  - seq_len==1 MHA reduces to out_proj(v_proj(kv)); fused host-side.  The
    self-attention residual is folded into a single matmul (I + Wv@Wo).
  - The cross-attention pair average (a+b)/2 is a concat matmul producing
    m' = m/2 directly (0.5 folded into the weights); the gate sigmoid is
    computed as 0.5*tanh(0.5*x)+0.5 so that gelu+tanh live in ONE scalar
    engine table set (no ACT table switches).
  - The n2 LayerNorm (post-gate) is folded into fus_W1 (gamma scales into the
    weights, per-sample mean correction as a rank-1 term, betas into bias).
  - Execution is a single software-pipelined stream of 34 matmul tile-groups
    (layer x stream x batch-tile).  Weights are DMA'd once and shared by both
    batch tiles.  Matmuls run m-outer so PSUM banks retire eagerly; each
    group's LN stats matmuls and finish chain are emitted inside LATER
    groups' matmul streams so the (in-order) PE queue never waits on the
    vector/scalar engines.
  - LN statistics accumulate into one shared PSUM bank (one [1,512] row per
    group at a distinct partition offset); 1/sqrt(var+eps) uses the int32
    magic + Newton-Raphson on PE-transposed [128,4] tiles in a second
    (time-shared) scratch PSUM bank.
"""
import os
import sys

sys.path.insert(0, "/opt/trn_rl_repo")

import numpy as np

import concourse.bass as bass
import concourse.tile as tile
from concourse import bacc, mybir
from concourse.bass import ts
from concourse.bass_utils import run_bass_kernel_spmd
from concourse.masks import make_identity

D = 512
HID = 1024
B = 8192
NCORES = 8
BL = B // NCORES          # rows per core
NT = BL // 512            # batch tiles per core (2)
EPS = 1e-5
MAGIC = 0x5F3759DF
F32 = mybir.dt.float32
I32 = mybir.dt.int32
FS = [10, 6, 15]          # logit dims per stream
NR_ITERS = int(os.environ.get("KERNEL_NR_ITERS", "2"))
MM_DT = {
    "fp16": mybir.dt.float16,
    "bf16": mybir.dt.bfloat16,
}[os.environ.get("KERNEL_MM_DTYPE", "fp16")]
ACT = mybir.ActivationFunctionType

F64 = np.float64

M_STREAMS = [(1, 2), (0, 2), (0, 1)]   # m_s averages cross-attn from (a, b)


# --------------------------------------------------------------------------
# Host-side weight folding
# --------------------------------------------------------------------------

def _center_cols(W, b):
    W = np.asarray(W, F64)
    b = np.asarray(b, F64)
    return W - W.mean(axis=1, keepdims=True), b - b.mean()


def fold_weights(inp):
    g = lambda k: np.asarray(inp[k], dtype=F64)
    out = {}

    w_hp, b_hp = [], []
    for s in range(3):
        W, b = _center_cols(g("hp_W")[s], g("hp_b")[s])
        w_hp.append(W)
        b_hp.append(b)
    out["w_hp"] = np.stack(w_hp)
    out["b_hp"] = np.stack(b_hp)
    out["g_hp"], out["be_hp"] = g("hp_g"), g("hp_be")

    mhaW, mhab = g("mha_in_W"), g("mha_in_b")
    moW, mob = g("mha_out_W"), g("mha_out_b")
    Wv0, bv0 = mhaW[0][:, 2 * D:], mhab[0][2 * D:]
    Wr, br = _center_cols(np.eye(D) + Wv0 @ moW[0], bv0 @ moW[0] + mob[0])
    out["w_r"], out["b_r"] = Wr, br
    out["g_n1"], out["be_n1"] = g("n1_g"), g("n1_be")

    Wj, bj = [None] * 4, [None] * 4
    for j in (1, 2, 3):
        Wv, bv = mhaW[j][:, 2 * D:], mhab[j][2 * D:]
        Wj[j] = Wv @ moW[j]
        bj[j] = bv @ moW[j] + mob[j]
    mods = [(1, 2), (1, 3), (2, 3)]
    w_m, b_m = [], []
    for s in range(3):
        ja, jb = mods[s]
        w_m.append(np.concatenate([0.5 * Wj[ja], 0.5 * Wj[jb]], axis=0))
        b_m.append(0.5 * (bj[ja] + bj[jb]))
    out["w_m"] = np.stack(w_m)
    out["b_m"] = np.stack(b_m)

    out["w_g"] = g("gate_W")
    out["b_g"] = g("gate_b")

    w_lp, b_lp = [], []
    for s, key in enumerate(["verb", "inst", "target"]):
        W, b = _center_cols(g(f"lp_W_{key}"), g(f"lp_b_{key}"))
        w_lp.append(W)
        b_lp.append(b)
    out["w_lp"] = w_lp
    out["b_lp"] = np.stack(b_lp)
    out["g_lp"], out["be_lp"] = g("lp_g"), g("lp_be")

    W1 = g("fus_W1")
    g2, be2 = g("n2_g"), g("n2_be")
    A1, negc = [], []
    bias_total = g("fus_b1").copy()
    for s in range(3):
        blk = W1[s * D:(s + 1) * D]
        A = g2[s][:, None] * blk
        c = blk.T @ g2[s]
        A1.append(A - A.mean(axis=1, keepdims=True))
        negc.append(-(c - c.mean()))
        bias_total += be2[s] @ blk
    L1 = []
    for s in range(3):
        off = 3 * D + s * (D // 2)
        blk = W1[off: off + D // 2]
        L1.append(blk - blk.mean(axis=1, keepdims=True))
    out["w_f1"] = np.stack(A1)
    out["negc_f1"] = np.stack(negc)
    out["w_f1l"] = np.stack(L1)
    out["b_f1"] = bias_total - bias_total.mean()
    out["g_f1"], out["be_f1"] = g("fus_g1"), g("fus_ge1")

    W2c, b2c = _center_cols(g("fus_W2"), g("fus_b2"))
    out["w_f2"], out["b_f2"] = W2c, b2c
    out["g_f2"], out["be_f2"] = g("fus_g2"), g("fus_ge2")
    return out


def _vec_pp(v, nk):
    """[.., nk*128] feature vector -> per-partition layout [.., 128, nk]."""
    v = np.asarray(v, np.float32)
    return np.ascontiguousarray(v.reshape(v.shape[:-1] + (nk, 128)).swapaxes(-1, -2))


def _inv_sq(g):
    g = np.asarray(g, F64)
    return np.where(np.abs(g) > 1e-6, 1.0 / (g * g), 0.0)


def _np_mm_dtype():
    if MM_DT == mybir.dt.bfloat16:
        import ml_dtypes
        return ml_dtypes.bfloat16
    return np.float16


def device_arrays(fw):
    """Gamma-folded weights -> dict of arrays matching the DRAM tensor decls."""
    mmt = _np_mm_dtype()
    mm = lambda v: np.ascontiguousarray(np.asarray(v, F64).astype(mmt))
    dev = {}

    # hp: fold g_hp into columns
    w = fw["w_hp"] * fw["g_hp"][:, None, :]
    dev["w_hp"] = mm(w.reshape(3, 8, 128, 512))
    dev["b_hp"] = _vec_pp(fw["b_hp"] * fw["g_hp"], 4)
    dev["be_hp"] = _vec_pp(fw["be_hp"], 4)

    # r: per-stream copy with n1_g folded
    w_r = np.stack([fw["w_r"] * fw["g_n1"][s][None, :] for s in range(3)])
    dev["w_r"] = mm(w_r.reshape(3, 4, 128, 512))
    dev["b_r"] = _vec_pp(fw["b_r"][None] * fw["g_n1"], 4)
    dev["be_n1"] = _vec_pp(fw["be_n1"], 4)

    # m': 0.5 folded into weights/bias
    dev["w_m"] = mm((0.5 * fw["w_m"]).reshape(3, 8, 128, 512))
    dev["b_m"] = _vec_pp(0.5 * fw["b_m"], 4)

    # gate: bottom half (m rows) x2 to undo m'; tanh bias halved
    w_g = fw["w_g"].copy()
    w_g[:, D:, :] *= 2.0
    dev["w_g"] = mm(w_g.reshape(3, 8, 128, 512))
    dev["b_g"] = _vec_pp(0.5 * fw["b_g"], 4)

    # lp: fold g_lp
    for s in range(3):
        dev[f"w_lp{s}"] = mm(fw["w_lp"][s] * fw["g_lp"][s][None, :])
    dev["b_lp"] = _vec_pp(fw["b_lp"] * fw["g_lp"], 2)
    dev["be_lp"] = _vec_pp(fw["be_lp"], 2)

    # f1: fold g_f1 into all producing terms; cat in src order
    gf1 = fw["g_f1"]
    f1l = fw["w_f1l"] * gf1[None, None, :]
    f1a = fw["w_f1"] * gf1[None, None, :]
    blocks = [f1l[0], f1l[1], f1l[2], f1a[2], f1a[1], f1a[0]]
    rows = [blk.reshape(-1, 128, 512) for blk in blocks]
    dev["w_f1cat"] = mm(np.concatenate(rows, axis=0))       # [18,128,512]
    dev["negc"] = mm((fw["negc_f1"] * gf1[None, :])[None])  # [1,3,512]
    dev["b_f1"] = _vec_pp(fw["b_f1"] * gf1, 4)
    dev["be_f1"] = _vec_pp(fw["be_f1"], 4)

    # f2: fold g_f2
    dev["w_f2"] = mm((fw["w_f2"] * fw["g_f2"][None, :]).reshape(4, 128, 512))
    dev["b_f2"] = _vec_pp(fw["b_f2"] * fw["g_f2"], 4)
    dev["be_f2"] = _vec_pp(fw["be_f2"], 4)

    # stats columns: 1/g^2 per chunk.  layout: hp(12) r(12) lp(6) f1(4) f2(4)
    # ones(1) -> [128, 39]
    cols = []
    for s in range(3):
        cols.append(_vec_pp(_inv_sq(fw["g_hp"][s]), 4))      # [128,4]
    for s in range(3):
        cols.append(_vec_pp(_inv_sq(fw["g_n1"][s]), 4))
    for s in range(3):
        cols.append(_vec_pp(_inv_sq(fw["g_lp"][s]), 2))
    cols.append(_vec_pp(_inv_sq(fw["g_f1"]), 4))
    cols.append(_vec_pp(_inv_sq(fw["g_f2"]), 4))
    cols.append(np.ones((128, 1), np.float32))
    dev["statw"] = np.ascontiguousarray(
        np.concatenate(cols, axis=1).astype(mmt))            # [128, 39]

    # nonzero-beta flags (compile-time specialization)
    dev["_be_nz"] = {
        "r": float(np.abs(fw["be_n1"]).max()) > 0,
        "f2": float(np.abs(fw["be_f2"]).max()) > 0,
    }
    return dev


STATCOL = {}
_base = 0
for _s in range(3):
    STATCOL[("hp", _s)] = _base; _base += 4
for _s in range(3):
    STATCOL[("r", _s)] = _base; _base += 4
for _s in range(3):
    STATCOL[("lp", _s)] = _base; _base += 2
STATCOL[("f1", 0)] = _base; _base += 4
STATCOL[("f2", 0)] = _base; _base += 4
ONES_COL = _base
NSTATW = _base + 1


# --------------------------------------------------------------------------
# Device program
# --------------------------------------------------------------------------

def emit_program(tc, io, be_nz):
    nc = tc.nc
    MF = MM_DT
    from contextlib import ExitStack
    ctx = ExitStack()

    P = lambda name, bufs, space="SBUF": ctx.enter_context(
        tc.tile_pool(name=name, bufs=bufs, space=space))

    const = P("const", 1)
    xpool = P("x", 2)          # [128,8,512] MF input tiles
    wpool = P("w", 12)         # [128,2,512] MF weight pair-chunks
    ltp = P("lt", 1)           # logits + lsg
    evp = P("ev", 11)          # [128,512] MF evictions
    sqp = P("sq", 7)           # [128,512] MF squares
    zp = P("z", 3)             # [128,512] MF z tiles (pre-gelu)
    yhp = P("yh", 4)           # [128,4,512] MF
    ep = P("e", 6)
    mp = P("m", 2)
    tp = P("t", 2)
    ztp = P("zt", 6)
    lp_ = P("l", 6)            # [128,2,512] MF
    hp_ = P("h", 1)
    op_ = P("o", 1)
    stp = P("stats_sb", 2)
    bcp = P("bc", 2)
    wrp = P("wrow", 1)
    mm_ps = P("mm_ps", 6, "PSUM")
    st_ps = P("st_ps", 1, "PSUM")
    sc_ps = P("sc_ps", 1, "PSUM")

    ident = const.tile([128, 128], F32)
    make_identity(nc, ident)

    def load(name, shape, rearr=None, dtype=F32):
        t = const.tile(shape, dtype, name=name)
        src = io[name]
        if rearr:
            src = src.rearrange(rearr)
        nc.sync.dma_start(t[:], src)
        return t

    statw = load("statw", [128, NSTATW], dtype=MF)
    negc = load("negc", [1, 3, 512], dtype=MF)
    b_hp = load("b_hp", [128, 3, 4], "s p c -> p s c")
    be_hp = load("be_hp", [128, 3, 4], "s p c -> p s c")
    b_r = load("b_r", [128, 3, 4], "s p c -> p s c")
    be_n1 = load("be_n1", [128, 3, 4], "s p c -> p s c")
    b_m = load("b_m", [128, 3, 4], "s p c -> p s c")
    b_g = load("b_g", [128, 3, 4], "s p c -> p s c")
    b_lp = load("b_lp", [128, 3, 2], "s p c -> p s c")
    be_lp = load("be_lp", [128, 3, 2], "s p c -> p s c")
    b_f1 = load("b_f1", [128, 4])
    be_f1 = load("be_f1", [128, 4])
    b_f2 = load("b_f2", [128, 4])
    be_f2 = load("be_f2", [128, 4])
    w_lp = [load(f"w_lp{s}", [FS[s], 256], dtype=MF) for s in range(3)]

    # logits: load + sigmoid(-as-tanh) for all 6 (stream, tile) up front
    lsg = {}
    for s in range(3):
        for t in range(NT):
            lt = ltp.tile([FS[s], 512], F32, name="lt", tag="lt", bufs=1)
            nc.sync.dma_start(lt[:], io[f"lT{s}"][:, ts(t, 512)])
            th = ltp.tile([FS[s], 512], F32, name="th", tag="th", bufs=1)
            nc.scalar.activation(th[:], lt[:], ACT.Tanh, scale=0.5)
            sg = ltp.tile([FS[s], 512], MF, name="sg", tag=f"sg{s}_{t}", bufs=1)
            nc.vector.tensor_scalar(sg[:], th[:], 0.5, 0.5,
                                    mybir.AluOpType.mult, mybir.AluOpType.add)
            lsg[(s, t)] = sg

    # shared PSUM stats bank; rows handed out sequentially
    stats = st_ps.tile([128, 512], F32, name="stats")
    scratch = sc_ps.tile([128, 512], F32, name="scratch")
    wrall = wrp.tile([1, 6, 512], MF, name="wrall")

    # ---------------- persistent state ----------------
    xc = {}           # (s,t) -> [128,8,512] input tile
    weights = {}      # key -> list of wpair tiles
    yh = {}           # (s,t) -> [128,4,512]
    e_t = {}
    m_t = {}
    t_t = {}
    zt_t = {}
    l_t = {}
    h_t = {}
    w_row = {}
    ev_store = {}
    sq_store = {}
    row_of = {}
    fin_ct = [0]
    next_row = [0]

    pending_stats = []
    pending_fins = []

    flip = [0]

    def alt():
        flip[0] ^= 1
        return flip[0]

    def evict(dst_ap, ps_ap, bias_col):
        """PSUM -> SBUF with bias add, alternating ACT/DVE."""
        if alt():
            nc.scalar.activation(dst_ap, ps_ap, ACT.Identity, bias=bias_col)
        else:
            nc.vector.tensor_scalar_add(dst_ap, ps_ap, bias_col)

    # ---------------- weight DMA ----------------
    WCFG = {
        "hp": ("w_hp", 8), "r": ("w_r", 4), "m": ("w_m", 8), "g": ("w_g", 8),
    }

    def ensure_w(kind, s):
        key = (kind, s)
        if key in weights:
            return
        if kind in WCFG:
            name, nk = WCFG[kind]
            dram = io[name][s]
        elif kind == "f1":
            dram, nk = io["w_f1cat"], 18
        elif kind == "f2":
            dram, nk = io["w_f2"], 4
        else:                      # lp: const-resident
            weights[key] = None
            return
        tiles = []
        for c0 in range(0, nk, 2):
            wc = wpool.tile([128, 2, 512], MF, name=f"w{kind}{s}_{c0}", tag="w")
            nc.sync.dma_start(wc[:], dram[c0:c0 + 2].rearrange("c p n -> p c n"))
            tiles.append(wc)
        weights[key] = tiles

    def wchunk(kind, s, ci, m):
        wc = weights[(kind, s)][ci // 2]
        return wc[:, ci % 2, ts(m, 128)]

    def ensure_x(s, t):
        if (s, t) in xc:
            return
        x = xpool.tile([128, 8, 512], MF, name=f"x{s}_{t}", tag="x")
        nc.sync.dma_start(
            x[:], io[f"xT{s}"][:, ts(t, 512)].rearrange("(c p) b -> p c b", p=128))
        xc[(s, t)] = x

    # ---------------- istd chain (Newton-Raphson rsqrt) ----------------
    def emit_istd(v_sb, parity):
        vT = scratch[:, parity * 4:parity * 4 + 4]
        for c in range(4):
            nc.tensor.transpose(vT[:, c:c + 1], v_sb[0:1, ts(c, 128)],
                                ident[0:1, 0:1])
        y = stp.tile([128, 4], F32, name="nr_y", tag="nr_y", bufs=2)
        t2 = stp.tile([128, 4], F32, name="nr_t", tag="nr_t", bufs=2)
        nc.vector.tensor_scalar(y[:].bitcast(I32), vT[:].bitcast(I32),
                                1, None, mybir.AluOpType.logical_shift_right)
        nc.vector.tensor_scalar(y[:].bitcast(I32), y[:].bitcast(I32),
                                -1, MAGIC, mybir.AluOpType.mult,
                                mybir.AluOpType.add)
        for _ in range(NR_ITERS):
            nc.vector.tensor_mul(t2[:], y[:], y[:])
            nc.vector.tensor_mul(t2[:], t2[:], vT[:])
            nc.vector.tensor_scalar(t2[:], t2[:], -0.5, 1.5,
                                    mybir.AluOpType.mult, mybir.AluOpType.add)
            nc.vector.tensor_mul(y[:], y[:], t2[:])
        rT = scratch[0:1, :]
        for c in range(4):
            nc.tensor.transpose(rT[0:1, ts(c, 128)], y[:, c:c + 1], ident)
        istd = stp.tile([1, 512], MF, name="istd", tag="istd", bufs=3)
        nc.vector.tensor_copy(istd[:], rT[:])
        return istd, rT

    def bcast(istd):
        bc = bcp.tile([128, 512], MF, name="bc")
        nc.gpsimd.partition_broadcast(bc[:], istd[:])
        return bc

    # ---------------- unit definitions ----------------
    def unit_srcs(kind, s, t):
        """Returns (nbanks, src list of (lhsT_fn, rhs_ap))."""
        if kind == "hp":
            x = xc[(s, t)]
            return 4, [(lambda m, ci=ci: wchunk("hp", s, ci, m),
                        x[:, ci, :]) for ci in range(8)]
        if kind == "r":
            y = yh[(s, t)]
            return 4, [(lambda m, ci=ci: wchunk("r", s, ci, m),
                        y[:, ci, :]) for ci in range(4)]
        if kind == "m":
            a, b = M_STREAMS[s]
            rhs = [e_t[(a, t)][:, c, :] for c in range(4)] + \
                  [e_t[(b, t)][:, c, :] for c in range(4)]
            return 4, [(lambda m, ci=ci: wchunk("m", s, ci, m), rhs[ci])
                       for ci in range(8)]
        if kind == "g":
            rhs = [e_t[(s, t)][:, c, :] for c in range(4)] + \
                  [m_t[(s, t)][:, c, :] for c in range(4)]
            return 4, [(lambda m, ci=ci: wchunk("g", s, ci, m), rhs[ci])
                       for ci in range(8)]
        if kind == "lp":
            return 2, [(lambda m: w_lp[s][:, ts(m, 128)], lsg[(s, t)][:])]
        if kind == "f1":
            srcs = []
            ci = 0
            for ls in range(3):
                for c in range(2):
                    srcs.append((lambda m, ci=ci: wchunk("f1", 0, ci, m),
                                 l_t[(ls, t)][:, c, :]))
                    ci += 1
            for zs in (2, 1, 0):
                for c in range(4):
                    srcs.append((lambda m, ci=ci: wchunk("f1", 0, ci, m),
                                 zt_t[(zs, t)][:, c, :]))
                    ci += 1
            for zs in (2, 1, 0):
                srcs.append((lambda m, zs=zs: negc[0:1, zs, ts(m, 128)],
                             w_row[(zs, t)]))
            return 4, srcs
        if kind == "f2":
            h = h_t[t]
            return 4, [(lambda m, ci=ci: wchunk("f2", 0, ci, m),
                        h[:, ci, :]) for ci in range(4)]
        raise KeyError(kind)

    ROWS = (0, 32, 64)

    def alloc_rows(n):
        """Rotating quadrant-aligned stats-bank rows (PE psum-out rule).
        Rows are transient: written at one host unit, read at the next."""
        out = tuple(ROWS[(next_row[0] + i) % 3] for i in range(n))
        next_row[0] += n
        return out

    # -------- per-unit eviction + elementwise (emitted inline) --------
    def emit_unit_tail(kind, s, t, m, ps, uid):
        """Evict bank m of unit uid and square if the unit has a LN."""
        if kind == "m":
            if (s, t) not in m_t:
                m_t[(s, t)] = mp.tile([128, 4, 512], MF, name="m_sb")
            evict(m_t[(s, t)][:, m, :], ps[:], b_m[:, s, m:m + 1])
            return
        if kind == "g":
            # fused eviction: tanh(0.5*x + 0.5*b_g); then t = e + q' + m'
            th = evp.tile([128, 512], MF, name="th")
            nc.scalar.activation(th[:], ps[:], ACT.Tanh, scale=0.5,
                                 bias=b_g[:, s, m:m + 1])
            if (s, t) not in t_t:
                t_t[(s, t)] = tp.tile([128, 4, 512], MF, name="t_sb")
            tsb = t_t[(s, t)]
            q = zp.tile([128, 512], MF, name="q")
            nc.vector.tensor_mul(q[:], th[:], m_t[(s, t)][:, m, :])
            nc.vector.tensor_add(q[:], q[:], m_t[(s, t)][:, m, :])
            nc.vector.tensor_add(tsb[:, m, :], e_t[(s, t)][:, m, :], q[:])
            sq = sqp.tile([128, 512], MF, name="sq")
            nc.vector.tensor_mul(sq[:], tsb[:, m, :], tsb[:, m, :])
            sq_store.setdefault(uid, []).append(sq)
            return
        bias = {"hp": lambda: b_hp[:, s, m:m + 1],
                "r": lambda: b_r[:, s, m:m + 1],
                "lp": lambda: b_lp[:, s, m:m + 1],
                "f1": lambda: b_f1[:, m:m + 1],
                "f2": lambda: b_f2[:, m:m + 1]}[kind]()
        ev = evp.tile([128, 512], MF, name="ev")
        evict(ev[:], ps[:], bias)
        ev_store.setdefault(uid, []).append(ev)
        sq = sqp.tile([128, 512], MF, name="sq")
        nc.vector.tensor_mul(sq[:], ev[:], ev[:])
        sq_store.setdefault(uid, []).append(sq)

    # -------- deferred stats (pure PE) --------
    def make_stats_cb(kind, s, t, uid):
        def cb():
            if kind == "g":
                rs, rq = alloc_rows(2)
                row_of[uid] = (rs, rq)
                tsb = t_t[(s, t)]
                for c in range(4):
                    nc.tensor.matmul(stats[rs:rs + 1, :],
                                     statw[:, ONES_COL:ONES_COL + 1],
                                     tsb[:, c, :], start=(c == 0), stop=(c == 3))
                sqs = sq_store.pop(uid)
                for c in range(4):
                    nc.tensor.matmul(stats[rq:rq + 1, :],
                                     statw[:, ONES_COL:ONES_COL + 1],
                                     sqs[c][:], start=(c == 0), stop=(c == 3))
            else:
                (r,) = alloc_rows(1)
                row_of[uid] = (r,)
                base = STATCOL[(kind, 0 if kind in ("f1", "f2") else s)]
                sqs = sq_store.pop(uid)
                n = len(sqs)
                for c in range(n):
                    nc.tensor.matmul(stats[r:r + 1, :],
                                     statw[:, base + c:base + c + 1],
                                     sqs[c][:], start=(c == 0), stop=(c == n - 1))
        return cb

    # -------- deferred finish chain --------
    def make_fin_cb(kind, s, t, uid):
        def cb():
            parity = fin_ct[0] % 2
            fin_ct[0] += 1
            rows = row_of[uid]
            dim = 256.0 if kind == "lp" else 512.0
            vblk = stp.tile([1, 3, 512], F32, name="vblk", tag="vblk", bufs=1)
            v = vblk[0:1, 0, :]
            if kind == "g":
                rs, rq = rows
                mu = vblk[0:1, 1, :]
                msq = vblk[0:1, 2, :]
                nc.vector.tensor_scalar(mu, stats[rs:rs + 1, :], 1.0 / dim,
                                        None, mybir.AluOpType.mult)
                nc.vector.tensor_scalar(v, stats[rq:rq + 1, :], 1.0 / dim,
                                        EPS, mybir.AluOpType.mult,
                                        mybir.AluOpType.add)
                nc.vector.tensor_mul(msq, mu, mu)
                nc.vector.tensor_sub(v, v, msq)
            else:
                (r,) = rows
                nc.vector.tensor_scalar(v, stats[r:r + 1, :], 1.0 / dim,
                                        EPS, mybir.AluOpType.mult,
                                        mybir.AluOpType.add)
            istd, rT = emit_istd(v, parity)
            if kind == "g":
                idx = s * NT + t
                nc.vector.tensor_mul(wrall[0:1, idx, :], mu, rT[:])
                w_row[(s, t)] = wrall[0:1, idx, :]
            bc = bcast(istd)
            if kind == "g":
                tsb = t_t.pop((s, t))
                zt = ztp.tile([128, 4, 512], MF, name="zt_sb")
                for c in range(4):
                    nc.gpsimd.tensor_mul(zt[:, c, :], tsb[:, c, :], bc[:])
                zt_t[(s, t)] = zt
                return
            evs = ev_store.pop(uid)
            if kind == "hp":
                out = yhp.tile([128, 4, 512], MF, name="yh_sb")
                for c in range(4):
                    z = zp.tile([128, 512], MF, name="z")
                    nc.vector.tensor_mul(z[:], evs[c][:], bc[:])
                    nc.scalar.activation(out[:, c, :], z[:], ACT.Gelu,
                                         bias=be_hp[:, s, c:c + 1])
                yh[(s, t)] = out
            elif kind == "r":
                out = ep.tile([128, 4, 512], MF, name="e_sb")
                for c in range(4):
                    eng = nc.gpsimd if c % 2 else nc.vector
                    eng.tensor_mul(out[:, c, :], evs[c][:], bc[:])
                    if be_nz["r"]:
                        nc.vector.tensor_scalar_add(out[:, c, :], out[:, c, :],
                                                    be_n1[:, s, c:c + 1])
                e_t[(s, t)] = out
            elif kind == "lp":
                out = lp_.tile([128, 2, 512], MF, name="l_sb")
                for c in range(2):
                    z = zp.tile([128, 512], MF, name="z")
                    nc.vector.tensor_mul(z[:], evs[c][:], bc[:])
                    nc.scalar.activation(out[:, c, :], z[:], ACT.Gelu,
                                         bias=be_lp[:, s, c:c + 1])
                l_t[(s, t)] = out
            elif kind == "f1":
                out = hp_.tile([128, 4, 512], MF, name="h_sb")
                for c in range(4):
                    z = zp.tile([128, 512], MF, name="z")
                    nc.vector.tensor_mul(z[:], evs[c][:], bc[:])
                    nc.scalar.activation(out[:, c, :], z[:], ACT.Gelu,
                                         bias=be_f1[:, c:c + 1])
                h_t[t] = out
            elif kind == "f2":
                out = op_.tile([128, 4, 512], MF, name="o_sb")
                for c in range(4):
                    eng = nc.gpsimd if c % 2 else nc.vector
                    eng.tensor_mul(out[:, c, :], evs[c][:], bc[:])
                    if be_nz["f2"]:
                        nc.vector.tensor_scalar_add(out[:, c, :], out[:, c, :],
                                                    be_f2[:, c:c + 1])
                nc.sync.dma_start(
                    io["outT"].rearrange("(c p) b -> p c b", p=128)[:, :, ts(t, 512)],
                    out[:])
        return cb

    # ---------------- the pipeline ----------------
    units = []
    for s in range(3):
        for t in range(NT):
            units.append(("lp", s, t))
    units += [("hp", 0, 0), ("hp", 0, 1), ("hp", 1, 0), ("hp", 1, 1),
              ("r", 0, 0), ("r", 0, 1), ("hp", 2, 0), ("hp", 2, 1),
              ("r", 1, 0), ("r", 1, 1), ("r", 2, 0), ("r", 2, 1)]
    for s in (2, 1, 0):
        for t in range(NT):
            units.append(("m", s, t))
        for t in range(NT):
            units.append(("g", s, t))
    for t in range(NT):
        units.append(("f1", 0, t))
    for t in range(NT):
        units.append(("f2", 0, t))

    ensure_x(0, 0)
    ensure_w("hp", 0)

    for j, (kind, s, t) in enumerate(units):
        for la in (j + 1, j + 2):
            if la < len(units):
                k2, s2, t2 = units[la]
                ensure_w(k2, s2)
                if k2 == "hp":
                    ensure_x(s2, t2)

        nbanks, srcs = unit_srcs(kind, s, t)
        uid = j
        last = len(srcs) - 1
        fins_done = False
        for m in range(nbanks):
            ps = mm_ps.tile([128, 512], F32, name="mm")
            for ci, (wf, rhs) in enumerate(srcs):
                nc.tensor.matmul(ps[:], wf(m), rhs,
                                 start=(ci == 0), stop=(ci == last))
            if m == 1:
                for cb in pending_stats:
                    cb()
                pending_stats.clear()
            if m == 2:
                for cb in pending_fins:
                    cb()
                pending_fins.clear()
                fins_done = True
            emit_unit_tail(kind, s, t, m, ps, uid)
        if not fins_done:
            for cb in pending_fins:
                cb()
            pending_fins.clear()
        if kind != "m":
            pending_stats.append(make_stats_cb(kind, s, t, uid))
            pending_fins.append(make_fin_cb(kind, s, t, uid))

    for cb in pending_stats:
        cb()
    pending_stats.clear()
    for cb in pending_fins:
        cb()
    pending_fins.clear()

    ctx.close()


def build_program(be_nz):
    nc = bacc.Bacc("TRN2", target_bir_lowering=False, debug=False,
                   num_devices=NCORES)
    io = {}

    def din(name, shape, dtype=F32):
        io[name] = nc.dram_tensor(name, list(shape), dtype,
                                  kind="ExternalInput").ap()

    for s in range(3):
        din(f"xT{s}", (HID, BL), dtype=MM_DT)
        din(f"lT{s}", (FS[s], BL))
    din("w_hp", (3, 8, 128, 512), dtype=MM_DT)
    din("w_r", (3, 4, 128, 512), dtype=MM_DT)
    din("w_m", (3, 8, 128, 512), dtype=MM_DT)
    din("w_g", (3, 8, 128, 512), dtype=MM_DT)
    for s in range(3):
        din(f"w_lp{s}", (FS[s], 256), dtype=MM_DT)
    din("w_f1cat", (18, 128, 512), dtype=MM_DT)
    din("negc", (1, 3, 512), dtype=MM_DT)
    din("w_f2", (4, 128, 512), dtype=MM_DT)
    din("statw", (128, NSTATW), dtype=MM_DT)
    din("b_hp", (3, 128, 4))
    din("be_hp", (3, 128, 4))
    din("b_r", (3, 128, 4))
    din("be_n1", (3, 128, 4))
    din("b_m", (3, 128, 4))
    din("b_g", (3, 128, 4))
    din("b_lp", (3, 128, 2))
    din("be_lp", (3, 128, 2))
    din("b_f1", (128, 4))
    din("be_f1", (128, 4))
    din("b_f2", (128, 4))
    din("be_f2", (128, 4))
    io["outT"] = nc.dram_tensor("outT", [D, BL], MM_DT,
                                kind="ExternalOutput").ap()

    with tile.TileContext(nc) as tc:
        emit_program(tc, io, be_nz)
    nc.compile()
    return nc


def make_in_maps(inputs):
    fw = fold_weights(inputs)
    dev = device_arrays(fw)
    be_nz = dev.pop("_be_nz")
    mmt = _np_mm_dtype()
    hidden = [np.asarray(inputs["verb_hidden"], np.float32),
              np.asarray(inputs["inst_hidden"], np.float32),
              np.asarray(inputs["target_hidden"], np.float32)]
    logits = [np.asarray(inputs["verb_logits"], np.float32),
              np.asarray(inputs["inst_logits"], np.float32),
              np.asarray(inputs["target_logits"], np.float32)]
    in_maps = []
    for core in range(NCORES):
        rows = slice(core * BL, (core + 1) * BL)
        m = dict(dev)
        for s in range(3):
            m[f"xT{s}"] = np.ascontiguousarray(hidden[s][rows].T.astype(mmt))
            m[f"lT{s}"] = np.ascontiguousarray(logits[s][rows].T)
        in_maps.append(m)
    return in_maps, be_nz


_NC_CACHE = None


def _run(inputs, **spmd_kwargs):
    global _NC_CACHE
    in_maps, be_nz = make_in_maps(inputs)
    if _NC_CACHE is None:
        _NC_CACHE = build_program(be_nz)
    nc = _NC_CACHE
    res = run_bass_kernel_spmd(nc, in_maps, list(range(NCORES)),
                               **spmd_kwargs)
    out = np.empty((B, D), dtype=np.float32)
    for core in range(NCORES):
        out[core * BL:(core + 1) * BL] = \
            res.results[core]["outT"].astype(np.float32).T
    return out, res


def kernel(**inputs) -> np.ndarray:
    return _run(inputs)[0]


def kernel_profiled(inputs, tmpdir=None):
    """Returns (out, BassKernelResults) with an NTFF-based profile."""
    return _run(inputs, trace=True, tmpdir=tmpdir)


# revision 23
# speedup vs baseline: 1.3202x; 1.1648x over previous
"""Trainium2 Bass kernel for nn_AttentionModule_7146825580577.

Strategy: pure data parallel over the batch dim (8192 rows -> 1024 rows per
core, 8 cores), weights replicated.

Device math (per core), feature-transposed layout (features on SBUF
partitions, batch on the free dim), fp16 matmul operands with fp32 PSUM
accumulation:

  - All LayerNorm gammas are folded into the producing matmul's columns
    host-side; the variance is recovered with per-chunk 1/gamma^2 columns in
    the stats matmul.  Column-centered weights make mean(y) == 0 so only
    sum(y^2) is needed.
  - seq_len==1 MHA reduces to out_proj(v_proj(kv)); fused host-side.  The
    self-attention residual is folded into a single matmul (I + Wv@Wo).
  - The cross-attention pair average (a+b)/2 is a concat matmul producing
    m' = m/2 directly (0.5 folded into the weights); the gate sigmoid is
    computed as 0.5*tanh(0.5*x)+0.5 so that gelu+tanh live in ONE scalar
    engine table set (no ACT table switches).
  - The n2 LayerNorm (post-gate) is folded into fus_W1 (gamma scales into the
    weights, per-sample mean correction as a rank-1 term, betas into bias).
  - Execution is a single software-pipelined stream of 34 matmul tile-groups
    (layer x stream x batch-tile).  Weights are DMA'd once and shared by both
    batch tiles.  Matmuls run m-outer so PSUM banks retire eagerly; each
    group's LN stats matmuls and finish chain are emitted inside LATER
    groups' matmul streams so the (in-order) PE queue never waits on the
    vector/scalar engines.
  - LN statistics accumulate into one shared PSUM bank (one [1,512] row per
    group at a distinct partition offset); 1/sqrt(var+eps) uses the int32
    magic + Newton-Raphson on PE-transposed [128,4] tiles in a second
    (time-shared) scratch PSUM bank.
"""
import os
import sys

sys.path.insert(0, "/opt/trn_rl_repo")

import numpy as np

import concourse.bass as bass
import concourse.tile as tile
from concourse import bacc, mybir
from concourse.bass import ts
from concourse.bass_utils import run_bass_kernel_spmd
from concourse.masks import make_identity

D = 512
HID = 1024
B = 8192
NCORES = 8
BL = B // NCORES          # rows per core
NT = BL // 512            # batch tiles per core (2)
EPS = 1e-5
MAGIC = 0x5F3759DF
F32 = mybir.dt.float32
I32 = mybir.dt.int32
FS = [10, 6, 15]          # logit dims per stream
NR_ITERS = int(os.environ.get("KERNEL_NR_ITERS", "2"))
MM_DT = {
    "fp16": mybir.dt.float16,
    "bf16": mybir.dt.bfloat16,
}[os.environ.get("KERNEL_MM_DTYPE", "fp16")]
ACT = mybir.ActivationFunctionType

F64 = np.float64

M_STREAMS = [(1, 2), (0, 2), (0, 1)]   # m_s averages cross-attn from (a, b)


# --------------------------------------------------------------------------
# Host-side weight folding
# --------------------------------------------------------------------------

def _center_cols(W, b):
    W = np.asarray(W, F64)
    b = np.asarray(b, F64)
    return W - W.mean(axis=1, keepdims=True), b - b.mean()


def fold_weights(inp):
    g = lambda k: np.asarray(inp[k], dtype=F64)
    out = {}

    w_hp, b_hp = [], []
    for s in range(3):
        W, b = _center_cols(g("hp_W")[s], g("hp_b")[s])
        w_hp.append(W)
        b_hp.append(b)
    out["w_hp"] = np.stack(w_hp)
    out["b_hp"] = np.stack(b_hp)
    out["g_hp"], out["be_hp"] = g("hp_g"), g("hp_be")

    mhaW, mhab = g("mha_in_W"), g("mha_in_b")
    moW, mob = g("mha_out_W"), g("mha_out_b")
    Wv0, bv0 = mhaW[0][:, 2 * D:], mhab[0][2 * D:]
    Wr, br = _center_cols(np.eye(D) + Wv0 @ moW[0], bv0 @ moW[0] + mob[0])
    out["w_r"], out["b_r"] = Wr, br
    out["g_n1"], out["be_n1"] = g("n1_g"), g("n1_be")

    Wj, bj = [None] * 4, [None] * 4
    for j in (1, 2, 3):
        Wv, bv = mhaW[j][:, 2 * D:], mhab[j][2 * D:]
        Wj[j] = Wv @ moW[j]
        bj[j] = bv @ moW[j] + mob[j]
    mods = [(1, 2), (1, 3), (2, 3)]
    w_m, b_m = [], []
    for s in range(3):
        ja, jb = mods[s]
        w_m.append(np.concatenate([0.5 * Wj[ja], 0.5 * Wj[jb]], axis=0))
        b_m.append(0.5 * (bj[ja] + bj[jb]))
    out["w_m"] = np.stack(w_m)
    out["b_m"] = np.stack(b_m)

    out["w_g"] = g("gate_W")
    out["b_g"] = g("gate_b")

    w_lp, b_lp = [], []
    for s, key in enumerate(["verb", "inst", "target"]):
        W, b = _center_cols(g(f"lp_W_{key}"), g(f"lp_b_{key}"))
        w_lp.append(W)
        b_lp.append(b)
    out["w_lp"] = w_lp
    out["b_lp"] = np.stack(b_lp)
    out["g_lp"], out["be_lp"] = g("lp_g"), g("lp_be")

    W1 = g("fus_W1")
    g2, be2 = g("n2_g"), g("n2_be")
    A1, negc = [], []
    bias_total = g("fus_b1").copy()
    for s in range(3):
        blk = W1[s * D:(s + 1) * D]
        A = g2[s][:, None] * blk
        c = blk.T @ g2[s]
        A1.append(A - A.mean(axis=1, keepdims=True))
        negc.append(-(c - c.mean()))
        bias_total += be2[s] @ blk
    L1 = []
    for s in range(3):
        off = 3 * D + s * (D // 2)
        blk = W1[off: off + D // 2]
        L1.append(blk - blk.mean(axis=1, keepdims=True))
    out["w_f1"] = np.stack(A1)
    out["negc_f1"] = np.stack(negc)
    out["w_f1l"] = np.stack(L1)
    out["b_f1"] = bias_total - bias_total.mean()
    out["g_f1"], out["be_f1"] = g("fus_g1"), g("fus_ge1")

    W2c, b2c = _center_cols(g("fus_W2"), g("fus_b2"))
    out["w_f2"], out["b_f2"] = W2c, b2c
    out["g_f2"], out["be_f2"] = g("fus_g2"), g("fus_ge2")
    return out


def _vec_pp(v, nk):
    """[.., nk*128] feature vector -> per-partition layout [.., 128, nk]."""
    v = np.asarray(v, np.float32)
    return np.ascontiguousarray(v.reshape(v.shape[:-1] + (nk, 128)).swapaxes(-1, -2))


def _inv_sq(g):
    g = np.asarray(g, F64)
    return np.where(np.abs(g) > 1e-6, 1.0 / (g * g), 0.0)


def _np_mm_dtype():
    if MM_DT == mybir.dt.bfloat16:
        import ml_dtypes
        return ml_dtypes.bfloat16
    return np.float16


def device_arrays(fw):
    """Gamma-folded weights -> dict of arrays matching the DRAM tensor decls."""
    mmt = _np_mm_dtype()
    mm = lambda v: np.ascontiguousarray(np.asarray(v, F64).astype(mmt))
    dev = {}

    # hp: fold g_hp into columns
    w = fw["w_hp"] * fw["g_hp"][:, None, :]
    dev["w_hp"] = mm(w.reshape(3, 8, 128, 512))
    dev["b_hp"] = _vec_pp(fw["b_hp"] * fw["g_hp"], 4)
    dev["be_hp"] = _vec_pp(fw["be_hp"], 4)

    # r: per-stream copy with n1_g folded
    w_r = np.stack([fw["w_r"] * fw["g_n1"][s][None, :] for s in range(3)])
    dev["w_r"] = mm(w_r.reshape(3, 4, 128, 512))
    dev["b_r"] = _vec_pp(fw["b_r"][None] * fw["g_n1"], 4)
    dev["be_n1"] = _vec_pp(fw["be_n1"], 4)

    # m': 0.5 folded into weights/bias
    dev["w_m"] = mm((0.5 * fw["w_m"]).reshape(3, 8, 128, 512))
    dev["b_m"] = _vec_pp(0.5 * fw["b_m"], 4)

    # gate: bottom half (m rows) x2 to undo m'; tanh bias halved
    w_g = fw["w_g"].copy()
    w_g[:, D:, :] *= 2.0
    dev["w_g"] = mm(w_g.reshape(3, 8, 128, 512))
    dev["b_g"] = _vec_pp(0.5 * fw["b_g"], 4)

    # lp: fold g_lp
    for s in range(3):
        dev[f"w_lp{s}"] = mm(fw["w_lp"][s] * fw["g_lp"][s][None, :])
    dev["b_lp"] = _vec_pp(fw["b_lp"] * fw["g_lp"], 2)
    dev["be_lp"] = _vec_pp(fw["be_lp"], 2)

    # f1: fold g_f1 into all producing terms; cat in src order
    gf1 = fw["g_f1"]
    f1l = fw["w_f1l"] * gf1[None, None, :]
    f1a = fw["w_f1"] * gf1[None, None, :]
    blocks = [f1l[0], f1l[1], f1l[2], f1a[2], f1a[1], f1a[0]]
    rows = [blk.reshape(-1, 128, 512) for blk in blocks]
    dev["w_f1cat"] = mm(np.concatenate(rows, axis=0))       # [18,128,512]
    dev["negc"] = mm((fw["negc_f1"] * gf1[None, :])[None])  # [1,3,512]
    dev["b_f1"] = _vec_pp(fw["b_f1"] * gf1, 4)
    dev["be_f1"] = _vec_pp(fw["be_f1"], 4)

    # f2: fold g_f2
    dev["w_f2"] = mm((fw["w_f2"] * fw["g_f2"][None, :]).reshape(4, 128, 512))
    dev["b_f2"] = _vec_pp(fw["b_f2"] * fw["g_f2"], 4)
    dev["be_f2"] = _vec_pp(fw["be_f2"], 4)

    # stats columns: 1/g^2 per chunk.  layout: hp(12) r(12) lp(6) f1(4) f2(4)
    # ones(1) -> [128, 39]
    cols = []
    for s in range(3):
        cols.append(_vec_pp(_inv_sq(fw["g_hp"][s]), 4))      # [128,4]
    for s in range(3):
        cols.append(_vec_pp(_inv_sq(fw["g_n1"][s]), 4))
    for s in range(3):
        cols.append(_vec_pp(_inv_sq(fw["g_lp"][s]), 2))
    cols.append(_vec_pp(_inv_sq(fw["g_f1"]), 4))
    cols.append(_vec_pp(_inv_sq(fw["g_f2"]), 4))
    cols.append(np.ones((128, 1), np.float32))
    dev["statw"] = np.ascontiguousarray(
        np.concatenate(cols, axis=1).astype(mmt))            # [128, 39]

    # nonzero-beta flags (compile-time specialization)
    dev["_be_nz"] = {
        "r": float(np.abs(fw["be_n1"]).max()) > 0,
        "f2": float(np.abs(fw["be_f2"]).max()) > 0,
    }
    return dev


STATCOL = {}
_base = 0
for _s in range(3):
    STATCOL[("hp", _s)] = _base; _base += 4
for _s in range(3):
    STATCOL[("r", _s)] = _base; _base += 4
for _s in range(3):
    STATCOL[("lp", _s)] = _base; _base += 2
STATCOL[("f1", 0)] = _base; _base += 4
STATCOL[("f2", 0)] = _base; _base += 4
ONES_COL = _base
NSTATW = _base + 1


# --------------------------------------------------------------------------
# Device program
# --------------------------------------------------------------------------

def emit_program(tc, io, be_nz):
    nc = tc.nc
    MF = MM_DT
    from contextlib import ExitStack
    ctx = ExitStack()

    P = lambda name, bufs, space="SBUF": ctx.enter_context(
        tc.tile_pool(name=name, bufs=bufs, space=space))

    const = P("const", 1)
    xpool = P("x", 2)          # [128,8,512] MF input tiles
    wpool = P("w", 12)         # [128,2,512] MF weight pair-chunks
    ltp = P("lt", 1)           # logits + lsg
    evp = P("ev", 11)          # [128,512] MF evictions
    sqp = P("sq", 7)           # [128,512] MF squares
    zp = P("z", 3)             # [128,512] MF z tiles (pre-gelu)
    yhp = P("yh", 4)           # [128,4,512] MF
    ep = P("e", 6)
    mp = P("m", 2)
    tp = P("t", 2)
    ztp = P("zt", 6)
    lp_ = P("l", 6)            # [128,2,512] MF
    hp_ = P("h", 1)
    op_ = P("o", 1)
    stp = P("stats_sb", 2)
    bcp = P("bc", 2)
    wrp = P("wrow", 1)
    mm_ps = P("mm_ps", 6, "PSUM")
    st_ps = P("st_ps", 1, "PSUM")
    sc_ps = P("sc_ps", 1, "PSUM")

    ident = const.tile([128, 128], F32)
    make_identity(nc, ident)
    epscol = const.tile([1, 1], F32, name="epscol")
    nc.vector.memset(epscol[:], EPS)

    def load(name, shape, rearr=None, dtype=F32):
        t = const.tile(shape, dtype, name=name)
        src = io[name]
        if rearr:
            src = src.rearrange(rearr)
        nc.sync.dma_start(t[:], src)
        return t

    statw = load("statw", [128, NSTATW], dtype=MF)
    negc = load("negc", [1, 3, 512], dtype=MF)
    b_hp = load("b_hp", [128, 3, 4], "s p c -> p s c")
    be_hp = load("be_hp", [128, 3, 4], "s p c -> p s c")
    b_r = load("b_r", [128, 3, 4], "s p c -> p s c")
    be_n1 = load("be_n1", [128, 3, 4], "s p c -> p s c")
    b_m = load("b_m", [128, 3, 4], "s p c -> p s c")
    b_g = load("b_g", [128, 3, 4], "s p c -> p s c")
    b_lp = load("b_lp", [128, 3, 2], "s p c -> p s c")
    be_lp = load("be_lp", [128, 3, 2], "s p c -> p s c")
    b_f1 = load("b_f1", [128, 4])
    be_f1 = load("be_f1", [128, 4])
    b_f2 = load("b_f2", [128, 4])
    be_f2 = load("be_f2", [128, 4])
    w_lp = [load(f"w_lp{s}", [FS[s], 256], dtype=MF) for s in range(3)]

    # logits: load + sigmoid(-as-tanh) for all 6 (stream, tile) up front
    lsg = {}
    for s in range(3):
        for t in range(NT):
            lt = ltp.tile([FS[s], 512], F32, name="lt", tag="lt", bufs=1)
            nc.sync.dma_start(lt[:], io[f"lT{s}"][:, ts(t, 512)])
            th = ltp.tile([FS[s], 512], F32, name="th", tag="th", bufs=1)
            nc.scalar.activation(th[:], lt[:], ACT.Tanh, scale=0.5)
            sg = ltp.tile([FS[s], 512], MF, name="sg", tag=f"sg{s}_{t}", bufs=1)
            nc.vector.tensor_scalar(sg[:], th[:], 0.5, 0.5,
                                    mybir.AluOpType.mult, mybir.AluOpType.add)
            lsg[(s, t)] = sg

    # shared PSUM stats bank; rows handed out sequentially
    stats = st_ps.tile([128, 512], F32, name="stats")
    scratch = sc_ps.tile([128, 512], F32, name="scratch")
    wrall = wrp.tile([1, 6, 512], MF, name="wrall")

    # ---------------- persistent state ----------------
    xc = {}           # (s,t) -> [128,8,512] input tile
    weights = {}      # key -> list of wpair tiles
    yh = {}           # (s,t) -> [128,4,512]
    e_t = {}
    m_t = {}
    t_t = {}
    zt_t = {}
    l_t = {}
    h_t = {}
    w_row = {}
    ev_store = {}
    sq_store = {}
    row_of = {}
    fin_ct = [0]
    next_row = [0]

    pending_stats = []
    pending_fins = []

    flip = [0]

    def alt():
        flip[0] ^= 1
        return flip[0]

    def evict(dst_ap, ps_ap, bias_col):
        """PSUM -> SBUF with bias add, alternating ACT/DVE."""
        if alt():
            nc.scalar.activation(dst_ap, ps_ap, ACT.Identity, bias=bias_col)
        else:
            nc.vector.tensor_scalar_add(dst_ap, ps_ap, bias_col)

    # ---------------- weight DMA ----------------
    WCFG = {
        "hp": ("w_hp", 8), "r": ("w_r", 4), "m": ("w_m", 8), "g": ("w_g", 8),
    }

    def ensure_w(kind, s):
        key = (kind, s)
        if key in weights:
            return
        if kind in WCFG:
            name, nk = WCFG[kind]
            dram = io[name][s]
        elif kind == "f1":
            dram, nk = io["w_f1cat"], 18
        elif kind == "f2":
            dram, nk = io["w_f2"], 4
        else:                      # lp: const-resident
            weights[key] = None
            return
        tiles = []
        for c0 in range(0, nk, 2):
            wc = wpool.tile([128, 2, 512], MF, name=f"w{kind}{s}_{c0}", tag="w")
            nc.sync.dma_start(wc[:], dram[c0:c0 + 2].rearrange("c p n -> p c n"))
            tiles.append(wc)
        weights[key] = tiles

    def wchunk(kind, s, ci, m):
        wc = weights[(kind, s)][ci // 2]
        return wc[:, ci % 2, ts(m, 128)]

    def ensure_x(s, t):
        if (s, t) in xc:
            return
        x = xpool.tile([128, 8, 512], MF, name=f"x{s}_{t}", tag="x")
        nc.sync.dma_start(
            x[:], io[f"xT{s}"][:, ts(t, 512)].rearrange("(c p) b -> p c b", p=128))
        xc[(s, t)] = x

    # ---------------- istd chain (Newton-Raphson rsqrt) ----------------
    def emit_istd(v_sb, parity):
        vT = scratch[:, parity * 4:parity * 4 + 4]
        for c in range(4):
            nc.tensor.transpose(vT[:, c:c + 1], v_sb[0:1, ts(c, 128)],
                                ident[0:1, 0:1])
        y = stp.tile([128, 4], F32, name="nr_y", tag="nr_y", bufs=2)
        t2 = stp.tile([128, 4], F32, name="nr_t", tag="nr_t", bufs=2)
        nc.vector.tensor_scalar(y[:].bitcast(I32), vT[:].bitcast(I32),
                                1, None, mybir.AluOpType.logical_shift_right)
        nc.vector.tensor_scalar(y[:].bitcast(I32), y[:].bitcast(I32),
                                -1, MAGIC, mybir.AluOpType.mult,
                                mybir.AluOpType.add)
        for _ in range(NR_ITERS):
            nc.vector.tensor_mul(t2[:], y[:], y[:])
            nc.vector.tensor_mul(t2[:], t2[:], vT[:])
            nc.vector.tensor_scalar(t2[:], t2[:], -0.5, 1.5,
                                    mybir.AluOpType.mult, mybir.AluOpType.add)
            nc.vector.tensor_mul(y[:], y[:], t2[:])
        rT = scratch[0:1, :]
        for c in range(4):
            nc.tensor.transpose(rT[0:1, ts(c, 128)], y[:, c:c + 1], ident)
        istd = stp.tile([1, 512], MF, name="istd", tag="istd", bufs=3)
        nc.vector.tensor_copy(istd[:], rT[:])
        return istd, rT

    def bcast(istd):
        bc = bcp.tile([128, 512], MF, name="bc")
        nc.gpsimd.partition_broadcast(bc[:], istd[:])
        return bc

    # ---------------- unit definitions ----------------
    def unit_srcs(kind, s, t):
        """Returns (nbanks, src list of (lhsT_fn, rhs_ap))."""
        if kind == "hp":
            x = xc[(s, t)]
            return 4, [(lambda m, ci=ci: wchunk("hp", s, ci, m),
                        x[:, ci, :]) for ci in range(8)]
        if kind == "r":
            y = yh[(s, t)]
            return 4, [(lambda m, ci=ci: wchunk("r", s, ci, m),
                        y[:, ci, :]) for ci in range(4)]
        if kind == "m":
            a, b = M_STREAMS[s]
            rhs = [e_t[(a, t)][:, c, :] for c in range(4)] + \
                  [e_t[(b, t)][:, c, :] for c in range(4)]
            return 4, [(lambda m, ci=ci: wchunk("m", s, ci, m), rhs[ci])
                       for ci in range(8)]
        if kind == "g":
            rhs = [e_t[(s, t)][:, c, :] for c in range(4)] + \
                  [m_t[(s, t)][:, c, :] for c in range(4)]
            return 4, [(lambda m, ci=ci: wchunk("g", s, ci, m), rhs[ci])
                       for ci in range(8)]
        if kind == "lp":
            return 2, [(lambda m: w_lp[s][:, ts(m, 128)], lsg[(s, t)][:])]
        if kind == "f1":
            srcs = []
            ci = 0
            for ls in range(3):
                for c in range(2):
                    srcs.append((lambda m, ci=ci: wchunk("f1", 0, ci, m),
                                 l_t[(ls, t)][:, c, :]))
                    ci += 1
            for zs in (2, 1, 0):
                for c in range(4):
                    srcs.append((lambda m, ci=ci: wchunk("f1", 0, ci, m),
                                 zt_t[(zs, t)][:, c, :]))
                    ci += 1
            for zs in (2, 1, 0):
                srcs.append((lambda m, zs=zs: negc[0:1, zs, ts(m, 128)],
                             w_row[(zs, t)]))
            return 4, srcs
        if kind == "f2":
            h = h_t[t]
            return 4, [(lambda m, ci=ci: wchunk("f2", 0, ci, m),
                        h[:, ci, :]) for ci in range(4)]
        raise KeyError(kind)

    ROWS = (0, 32, 64)

    def alloc_rows(n):
        """Rotating quadrant-aligned stats-bank rows (PE psum-out rule).
        Rows are transient: written at one host unit, read at the next."""
        out = tuple(ROWS[(next_row[0] + i) % 3] for i in range(n))
        next_row[0] += n
        return out

    # -------- per-unit eviction + elementwise (emitted inline) --------
    def emit_unit_tail(kind, s, t, m, ps, uid):
        """Evict bank m of unit uid and square if the unit has a LN."""
        if kind == "m":
            if (s, t) not in m_t:
                m_t[(s, t)] = mp.tile([128, 4, 512], MF, name="m_sb")
            evict(m_t[(s, t)][:, m, :], ps[:], b_m[:, s, m:m + 1])
            return
        if kind == "g":
            # fused eviction: tanh(0.5*x + 0.5*b_g); then t = e + q' + m'
            th = evp.tile([128, 512], MF, name="th")
            nc.scalar.activation(th[:], ps[:], ACT.Tanh, scale=0.5,
                                 bias=b_g[:, s, m:m + 1])
            if (s, t) not in t_t:
                t_t[(s, t)] = tp.tile([128, 4, 512], MF, name="t_sb")
            tsb = t_t[(s, t)]
            q = zp.tile([128, 512], MF, name="q")
            nc.vector.scalar_tensor_tensor(q[:], th[:], 1.0, m_t[(s, t)][:, m, :],
                                           mybir.AluOpType.add,
                                           mybir.AluOpType.mult)
            nc.vector.tensor_add(tsb[:, m, :], e_t[(s, t)][:, m, :], q[:])
            sq = sqp.tile([128, 512], MF, name="sq")
            nc.vector.tensor_mul(sq[:], tsb[:, m, :], tsb[:, m, :])
            sq_store.setdefault(uid, []).append(sq)
            return
        bias = {"hp": lambda: b_hp[:, s, m:m + 1],
                "r": lambda: b_r[:, s, m:m + 1],
                "lp": lambda: b_lp[:, s, m:m + 1],
                "f1": lambda: b_f1[:, m:m + 1],
                "f2": lambda: b_f2[:, m:m + 1]}[kind]()
        ev = evp.tile([128, 512], MF, name="ev")
        evict(ev[:], ps[:], bias)
        ev_store.setdefault(uid, []).append(ev)
        sq = sqp.tile([128, 512], MF, name="sq")
        nc.vector.tensor_mul(sq[:], ev[:], ev[:])
        sq_store.setdefault(uid, []).append(sq)

    # -------- deferred stats (pure PE) --------
    def make_stats_cb(kind, s, t, uid):
        def cb():
            if kind == "g":
                rs, rq = alloc_rows(2)
                row_of[uid] = (rs, rq)
                tsb = t_t[(s, t)]
                for c in range(4):
                    nc.tensor.matmul(stats[rs:rs + 1, :],
                                     statw[:, ONES_COL:ONES_COL + 1],
                                     tsb[:, c, :], start=(c == 0), stop=(c == 3))
                sqs = sq_store.pop(uid)
                for c in range(4):
                    nc.tensor.matmul(stats[rq:rq + 1, :],
                                     statw[:, ONES_COL:ONES_COL + 1],
                                     sqs[c][:], start=(c == 0), stop=(c == 3))
            else:
                (r,) = alloc_rows(1)
                row_of[uid] = (r,)
                base = STATCOL[(kind, 0 if kind in ("f1", "f2") else s)]
                sqs = sq_store.pop(uid)
                n = len(sqs)
                for c in range(n):
                    nc.tensor.matmul(stats[r:r + 1, :],
                                     statw[:, base + c:base + c + 1],
                                     sqs[c][:], start=(c == 0), stop=(c == n - 1))
        return cb

    # -------- deferred finish chain --------
    def make_fin_cb(kind, s, t, uid):
        def cb():
            parity = fin_ct[0] % 2
            fin_ct[0] += 1
            rows = row_of[uid]
            dim = 256.0 if kind == "lp" else 512.0
            vblk = stp.tile([1, 3, 512], F32, name="vblk", tag="vblk", bufs=1)
            v = vblk[0:1, 0, :]
            if kind == "g":
                rs, rq = rows
                mu = vblk[0:1, 1, :]
                msq = vblk[0:1, 2, :]
                nc.scalar.activation(mu, stats[rs:rs + 1, :], ACT.Identity,
                                     scale=1.0 / dim)
                nc.scalar.activation(v, stats[rq:rq + 1, :], ACT.Identity,
                                     bias=epscol[0:1, 0:1], scale=1.0 / dim)
                nc.vector.tensor_mul(msq, mu, mu)
                nc.vector.tensor_sub(v, v, msq)
            else:
                (r,) = rows
                nc.scalar.activation(v, stats[r:r + 1, :], ACT.Identity,
                                     bias=epscol[0:1, 0:1], scale=1.0 / dim)
            istd, rT = emit_istd(v, parity)
            if kind == "g":
                idx = s * NT + t
                nc.vector.tensor_mul(wrall[0:1, idx, :], mu, rT[:])
                w_row[(s, t)] = wrall[0:1, idx, :]
            bc = bcast(istd)
            if kind == "g":
                tsb = t_t.pop((s, t))
                zt = ztp.tile([128, 4, 512], MF, name="zt_sb")
                for c in range(4):
                    nc.vector.tensor_mul(zt[:, c, :], tsb[:, c, :], bc[:])
                zt_t[(s, t)] = zt
                return
            evs = ev_store.pop(uid)
            if kind == "hp":
                out = yhp.tile([128, 4, 512], MF, name="yh_sb")
                for c in range(4):
                    z = zp.tile([128, 512], MF, name="z")
                    nc.vector.tensor_mul(z[:], evs[c][:], bc[:])
                    nc.scalar.activation(out[:, c, :], z[:], ACT.Gelu,
                                         bias=be_hp[:, s, c:c + 1])
                yh[(s, t)] = out
            elif kind == "r":
                out = ep.tile([128, 4, 512], MF, name="e_sb")
                for c in range(4):
                    nc.vector.tensor_mul(out[:, c, :], evs[c][:], bc[:])
                    if be_nz["r"]:
                        nc.vector.tensor_scalar_add(out[:, c, :], out[:, c, :],
                                                    be_n1[:, s, c:c + 1])
                e_t[(s, t)] = out
            elif kind == "lp":
                out = lp_.tile([128, 2, 512], MF, name="l_sb")
                for c in range(2):
                    z = zp.tile([128, 512], MF, name="z")
                    nc.vector.tensor_mul(z[:], evs[c][:], bc[:])
                    nc.scalar.activation(out[:, c, :], z[:], ACT.Gelu,
                                         bias=be_lp[:, s, c:c + 1])
                l_t[(s, t)] = out
            elif kind == "f1":
                out = hp_.tile([128, 4, 512], MF, name="h_sb")
                for c in range(4):
                    z = zp.tile([128, 512], MF, name="z")
                    nc.vector.tensor_mul(z[:], evs[c][:], bc[:])
                    nc.scalar.activation(out[:, c, :], z[:], ACT.Gelu,
                                         bias=be_f1[:, c:c + 1])
                h_t[t] = out
            elif kind == "f2":
                out = op_.tile([128, 4, 512], MF, name="o_sb")
                for c in range(4):
                    nc.vector.tensor_mul(out[:, c, :], evs[c][:], bc[:])
                    if be_nz["f2"]:
                        nc.vector.tensor_scalar_add(out[:, c, :], out[:, c, :],
                                                    be_f2[:, c:c + 1])
                nc.sync.dma_start(
                    io["outT"].rearrange("(c p) b -> p c b", p=128)[:, :, ts(t, 512)],
                    out[:])
        return cb

    # ---------------- the pipeline ----------------
    units = []
    for s in range(3):
        for t in range(NT):
            units.append(("lp", s, t))
    units += [("hp", 0, 0), ("hp", 0, 1), ("hp", 1, 0), ("hp", 1, 1),
              ("r", 0, 0), ("r", 0, 1), ("hp", 2, 0), ("hp", 2, 1),
              ("r", 1, 0), ("r", 1, 1), ("r", 2, 0), ("r", 2, 1)]
    for s in (2, 1, 0):
        for t in range(NT):
            units.append(("m", s, t))
        for t in range(NT):
            units.append(("g", s, t))
    for t in range(NT):
        units.append(("f1", 0, t))
    for t in range(NT):
        units.append(("f2", 0, t))

    ensure_x(0, 0)
    ensure_w("hp", 0)

    for j, (kind, s, t) in enumerate(units):
        for la in (j + 1, j + 2):
            if la < len(units):
                k2, s2, t2 = units[la]
                ensure_w(k2, s2)
                if k2 == "hp":
                    ensure_x(s2, t2)

        nbanks, srcs = unit_srcs(kind, s, t)
        uid = j
        last = len(srcs) - 1
        fins_done = False
        for m in range(nbanks):
            ps = mm_ps.tile([128, 512], F32, name="mm")
            for ci, (wf, rhs) in enumerate(srcs):
                nc.tensor.matmul(ps[:], wf(m), rhs,
                                 start=(ci == 0), stop=(ci == last))
            if m == 1:
                for cb in pending_stats:
                    cb()
                pending_stats.clear()
            if m == 2:
                for cb in pending_fins:
                    cb()
                pending_fins.clear()
                fins_done = True
            emit_unit_tail(kind, s, t, m, ps, uid)
        if not fins_done:
            for cb in pending_fins:
                cb()
            pending_fins.clear()
        if kind != "m":
            pending_stats.append(make_stats_cb(kind, s, t, uid))
            pending_fins.append(make_fin_cb(kind, s, t, uid))

    for cb in pending_stats:
        cb()
    pending_stats.clear()
    for cb in pending_fins:
        cb()
    pending_fins.clear()

    ctx.close()


def build_program(be_nz):
    nc = bacc.Bacc("TRN2", target_bir_lowering=False, debug=False,
                   num_devices=NCORES)
    io = {}

    def din(name, shape, dtype=F32):
        io[name] = nc.dram_tensor(name, list(shape), dtype,
                                  kind="ExternalInput").ap()

    for s in range(3):
        din(f"xT{s}", (HID, BL), dtype=MM_DT)
        din(f"lT{s}", (FS[s], BL))
    din("w_hp", (3, 8, 128, 512), dtype=MM_DT)
    din("w_r", (3, 4, 128, 512), dtype=MM_DT)
    din("w_m", (3, 8, 128, 512), dtype=MM_DT)
    din("w_g", (3, 8, 128, 512), dtype=MM_DT)
    for s in range(3):
        din(f"w_lp{s}", (FS[s], 256), dtype=MM_DT)
    din("w_f1cat", (18, 128, 512), dtype=MM_DT)
    din("negc", (1, 3, 512), dtype=MM_DT)
    din("w_f2", (4, 128, 512), dtype=MM_DT)
    din("statw", (128, NSTATW), dtype=MM_DT)
    din("b_hp", (3, 128, 4))
    din("be_hp", (3, 128, 4))
    din("b_r", (3, 128, 4))
    din("be_n1", (3, 128, 4))
    din("b_m", (3, 128, 4))
    din("b_g", (3, 128, 4))
    din("b_lp", (3, 128, 2))
    din("be_lp", (3, 128, 2))
    din("b_f1", (128, 4))
    din("be_f1", (128, 4))
    din("b_f2", (128, 4))
    din("be_f2", (128, 4))
    io["outT"] = nc.dram_tensor("outT", [D, BL], MM_DT,
                                kind="ExternalOutput").ap()

    with tile.TileContext(nc) as tc:
        emit_program(tc, io, be_nz)
    nc.compile()
    return nc


def make_in_maps(inputs):
    fw = fold_weights(inputs)
    dev = device_arrays(fw)
    be_nz = dev.pop("_be_nz")
    mmt = _np_mm_dtype()
    hidden = [np.asarray(inputs["verb_hidden"], np.float32),
              np.asarray(inputs["inst_hidden"], np.float32),
              np.asarray(inputs["target_hidden"], np.float32)]
    logits = [np.asarray(inputs["verb_logits"], np.float32),
              np.asarray(inputs["inst_logits"], np.float32),
              np.asarray(inputs["target_logits"], np.float32)]
    in_maps = []
    for core in range(NCORES):
        rows = slice(core * BL, (core + 1) * BL)
        m = dict(dev)
        for s in range(3):
            m[f"xT{s}"] = np.ascontiguousarray(hidden[s][rows].T.astype(mmt))
            m[f"lT{s}"] = np.ascontiguousarray(logits[s][rows].T)
        in_maps.append(m)
    return in_maps, be_nz


_NC_CACHE = None


def _run(inputs, **spmd_kwargs):
    global _NC_CACHE
    in_maps, be_nz = make_in_maps(inputs)
    if _NC_CACHE is None:
        _NC_CACHE = build_program(be_nz)
    nc = _NC_CACHE
    res = run_bass_kernel_spmd(nc, in_maps, list(range(NCORES)),
                               **spmd_kwargs)
    out = np.empty((B, D), dtype=np.float32)
    for core in range(NCORES):
        out[core * BL:(core + 1) * BL] = \
            res.results[core]["outT"].astype(np.float32).T
    return out, res


def kernel(**inputs) -> np.ndarray:
    return _run(inputs)[0]


def kernel_profiled(inputs, tmpdir=None):
    """Returns (out, BassKernelResults) with an NTFF-based profile."""
    return _run(inputs, trace=True, tmpdir=tmpdir)
